# revision 33
# baseline (speedup 1.0000x reference)
"""GAT encoder (3-layer) on 8 Trainium2 NeuronCores — scatter-free design.

Sharding: nodes partitioned across cores (graph partition). Edges partitioned
by destination node; weights replicated.

Key design vs the earlier gather+scatter version: the HW profile showed the
GpSimd Q7 core serially generating DMA descriptors for dma_gather AND
dma_scatter_add (~30us per 2048-edge chunk) while the DMA engines idled at
~50%. This version removes the scatter entirely and shrinks the table build:

  1. Per layer, each core computes Wh for ITS OWN nodes only (49 matmuls)
     and the AllGather of those [NLOC, F] bf16 shards IS the gather table
     (node-major [NPAD, F] in shared DRAM). No redundant 392-tile build,
     no separate table store, bf16 rows halve gather bytes.
  2. Edges are grouped by destination TILE (128 consecutive dst nodes), one
     chunk per tile. Segment softmax + scatter-add happen ON-CHIP: a DVE
     iota-compare builds S'[token, seg] = p_token one-hot-weighted, and
     16 PE matmuls accumulate out[seg, :] += S'_g.T @ gbuf_g in PSUM
     (fp32 accumulate). Sum-of-p comes from S'_g.T @ ones. Post-processing
     (divide, bias, ELU, transpose/pool) runs per chunk from PSUM — the
     out_aug HBM round-trip is gone.
  3. Gather idx streams carry trailing -1 pads (trimmed by Q7 before
     descriptor gen, so per-core count variance costs nothing); in-stream
     pads are killed by segid=-1 (S' row = 0) and alpha_d sentinel -1e9.
  4. Gathers cycle over 4 SWDGE queues so one chunk's drain overlaps the
     next chunk's descriptor generation.
"""

import math
import numpy as np

# ---------------- constants (hardcoded problem shape) ----------------
N = 50000
F = 128
G = 64
NCORES = 8
NLOC = 6272                   # 49*128 nodes per core (padded)
NPAD = NLOC * NCORES          # 50176
NTILES = NLOC // 128          # 49 = dst tiles per core = chunks per layer
BANK = 32768                  # gather bank split (int16 idx range)
NAUG = NLOC + 64              # alpha_d replicated width (sentinel tail)
NEG_SLOPE = 0.2
BIG_NEG = -1.0e9
EPS = 1.0e-16
IC_GROUP = 2                  # chunks per indirect-copy call (ISA dst limit 512)
PADFILL = True                # pad gather idx streams with 0 to full width


# ---------------- host-side preprocessing ----------------

def _build_edge_data(src, dst):
    """Group edges by (core, dst-tile, src-bank); build per-chunk gather idx
    streams (wrapped-16, trailing -1 pads), segid arrays, and the alpha_d
    indirect-copy idx stream."""
    per = {}
    for r in range(NCORES):
        lo, hi = r * NLOC, (r + 1) * NLOC
        m = (dst >= lo) & (dst < hi)
        gs = src[m].astype(np.int64)
        ld = (dst[m] - lo).astype(np.int64)
        tile = ld // 128
        seg = ld % 128
        bankB = gs >= BANK
        for j in range(NTILES):
            tm = tile == j
            mA = tm & ~bankB
            mB = tm & bankB
            per[(r, j)] = (gs[mA], seg[mA], gs[mB] - BANK, seg[mB])

    # per-chunk bank widths (max over cores, rounded to 16)
    CA = np.zeros(NTILES, np.int64)
    CB = np.zeros(NTILES, np.int64)
    for j in range(NTILES):
        for r in range(NCORES):
            gA, _, gB, _ = per[(r, j)]
            CA[j] = max(CA[j], len(gA))
            CB[j] = max(CB[j], len(gB))
    CA = ((CA + 15) // 16) * 16
    CB = ((CB + 15) // 16) * 16
    CACOLS = (CA + 127) // 128
    CBCOLS = (CB + 127) // 128
    if PADFILL:
        CA = CACOLS * 128
        CB = CBCOLS * 128
    COLS = CACOLS + CBCOLS                      # slot cols per chunk
    OFF = np.concatenate([[0], np.cumsum(COLS)])  # cumulative col offsets
    GSW = (CA + CB) // 16                        # idx words per chunk
    GSOFF = np.concatenate([[0], np.cumsum(GSW)])

    def wrap16(vals, width):
        """vals (int) -> [16, width//16] wrapped (token t -> [t%16, t//16]),
        then tiled to [128, width//16]."""
        a = np.full(width, 0 if PADFILL else -1, np.int64)
        a[:len(vals)] = vals
        t = np.arange(width)
        w = np.zeros((16, width // 16), np.int16)
        w[t % 16, t // 16] = a.astype(np.int16)
        return np.tile(w, (8, 1))

    # adt IC stream is padded to a fixed ICC=16 cols (2048 slots) per chunk
    # so the grouped indirect copies keep the known-good 512-elem geometry.
    ICC = 16
    assert COLS.max() <= ICC
    gsi = np.zeros((NCORES, 128, GSOFF[-1]), np.int16)
    segid = np.full((NCORES, 128, OFF[-1]), -1.0, np.float32)
    ld_tok = np.full((NCORES, NTILES * ICC * 128), NLOC, np.int64)

    for r in range(NCORES):
        for j in range(NTILES):
            gA, sA, gB, sB = per[(r, j)]
            gsi[r, :, GSOFF[j]:GSOFF[j] + CA[j] // 16] = wrap16(gA, CA[j])
            gsi[r, :, GSOFF[j] + CA[j] // 16:GSOFF[j + 1]] = wrap16(gB, CB[j])
            # slot s (= col*128 + p) -> seg / local dst
            base = j * ICC * 128
            for (g, s, s0) in ((gA, sA, 0), (gB, sB, 128 * CACOLS[j])):
                tt = s0 + np.arange(len(g))
                segid[r, tt % 128, OFF[j] + tt // 128] = s.astype(np.float32)
                ld_tok[r, base + tt] = j * 128 + s
    return dict(per=per, CA=CA, CB=CB, CACOLS=CACOLS, CBCOLS=CBCOLS,
                COLS=COLS, OFF=OFF, GSW=GSW, GSOFF=GSOFF,
                ICC=ICC), gsi, segid, ld_tok


def _ic_groups(n_chunks, icc, ic_limit=512):
    """Group consecutive chunks for the alpha_d indirect copy; each group
    covers gsz chunks of icc cols each, 16*gsz*icc <= ic_limit."""
    per = ic_limit // (16 * icc)
    groups = []
    pos = 0
    while pos < n_chunks:
        sz = min(per, n_chunks - pos)
        groups.append((pos, sz, sz * icc))
        pos += sz
    return groups


def _build_aidx(ld_tok_r, meta, groups):
    """Build the u16 idx stream for the grouped indirect copies.
    Group covering chunks [c0, c0+gsz) has C_all = gsz*ICC columns;
    IC output stream position i on partition 16*gg + (i%16), col i//16 maps
    to token tt = j*128 + 16*gg + k where k = i//C_all, j = i%C_all
    (each 16-partition group gg covers tokens with residue [16gg, 16gg+16))."""
    icc = meta["ICC"]
    parts = []
    for (c0, gsz, C_all) in groups:
        M = 16 * C_all
        out = np.zeros((128, M // 16), np.uint16)
        i_arr = np.arange(M)
        k_arr = i_arr // C_all
        j_arr = i_arr % C_all
        base = c0 * icc * 128
        ld = ld_tok_r[base:base + 128 * C_all]
        for gg in range(8):
            tt = j_arr * 128 + 16 * gg + k_arr
            out[16 * gg + i_arr % 16, i_arr // 16] = ld[tt].astype(np.uint16)
        parts.append(out)
    return np.concatenate(parts, axis=1)


def _prep_inputs(x, edge_index, batch, Ws, asrcs, adsts, bs):
    src = np.concatenate([edge_index[0], np.arange(N, dtype=np.int64)])
    dst = np.concatenate([edge_index[1], np.arange(N, dtype=np.int64)])
    src = np.asarray(src, np.int64)
    dst = np.asarray(dst, np.int64)

    meta, gsi, segid, ld_tok = _build_edge_data(src, dst)
    groups = _ic_groups(NTILES, meta["ICC"])
    meta["groups"] = groups

    xT_own = np.zeros((NCORES, F, NLOC), np.float32)
    xf = np.asarray(x, np.float32).T
    for r in range(NCORES):
        lo = r * NLOC
        w = min(NLOC, max(0, N - lo))
        xT_own[r, :, :w] = xf[:, lo:lo + w]

    w_aug = np.zeros((3, F, F + 2), np.float32)
    for k in range(3):
        w_aug[k, :, :F] = Ws[k]
        w_aug[k, :, F] = Ws[k] @ asrcs[k]
        w_aug[k, :, F + 1] = Ws[k] @ adsts[k]

    b_rep = np.zeros((3, 128, F), np.float32)
    for k in range(3):
        b_rep[k] = np.tile(bs[k][None, :], (128, 1))

    iota = np.tile(np.arange(128, dtype=np.float32)[None, :], (128, 1))

    batch64 = np.asarray(batch, np.int64)
    phot = np.zeros((NCORES, NTILES, 128, G), np.float32)
    for r in range(NCORES):
        base = r * NLOC
        for j in range(NTILES):
            nodes = base + j * 128 + np.arange(128)
            valid = nodes < N
            gsel = batch64[np.minimum(nodes, N - 1)]
            ph = np.zeros((128, G), np.float32)
            ph[np.arange(128)[valid], gsel[valid]] = 1.0
            phot[r, j] = ph

    counts = np.bincount(batch64, minlength=G).astype(np.float32)

    in_maps = []
    for r in range(NCORES):
        in_maps.append({
            "xT_own": np.ascontiguousarray(xT_own[r]),
            "w_aug": w_aug,
            "b_rep": b_rep,
            "gsi": np.ascontiguousarray(gsi[r]),
            "segid": np.ascontiguousarray(segid[r]),
            "aidx": _build_aidx(ld_tok[r], meta, groups),
            "iota": iota,
            "phot": phot[r].reshape(NTILES * 128, G),
        })
    return in_maps, meta, counts


# ---------------- numpy emulation of the device program ----------------

def _emulate_full(in_maps, meta, counts):
    CA, CB = meta["CA"], meta["CB"]
    CACOLS, COLS, OFF, GSOFF = meta["CACOLS"], meta["COLS"], meta["OFF"], meta["GSOFF"]
    hT_cur = [im["xT_own"].copy() for im in in_maps]
    pool_part = [np.zeros((G, F), np.float32) for _ in range(NCORES)]
    for k in range(3):
        # table = allgather of own Wh
        tabs = []
        alss = []
        ad_reps = []
        for r in range(NCORES):
            w = in_maps[r]["w_aug"][k]
            tabs.append((hT_cur[r].T @ w[:, :F]).astype(np.float32))
            alss.append((hT_cur[r].T @ w[:, F]).astype(np.float32))
            ad = (w[:, F + 1][None, :] @ hT_cur[r])[0]
            ad_aug = np.full(NAUG, BIG_NEG, np.float32)
            ad_aug[:NLOC] = ad
            ad_reps.append(ad_aug)
        table = np.concatenate(tabs, axis=0)          # [NPAD, F]
        als_tab = np.concatenate(alss, axis=0)        # [NPAD]
        new_hT = []
        for r in range(NCORES):
            im = in_maps[r]
            b = im["b_rep"][k][0]
            # adt via the aidx emulation (validates _build_aidx)
            ICC = meta["ICC"]
            adt_all = np.zeros((128, NTILES * ICC), np.float32)
            aoff = 0
            for (c0, gsz, C_all) in meta["groups"]:
                M = 16 * C_all
                i_arr = np.arange(M)
                k_arr = i_arr // C_all
                j_arr = i_arr % C_all
                a16 = im["aidx"][:, aoff:aoff + C_all].astype(np.int64)
                aoff += C_all
                for gg in range(8):
                    stream = a16[16 * gg + i_arr % 16, i_arr // 16]
                    vals = ad_reps[r][stream]
                    # stream pos i -> token (j*128 + 16gg + k) of group slots
                    tt = j_arr * 128 + 16 * gg + k_arr
                    adt_all[tt % 128, ICC * c0 + tt // 128] = vals
            ho_all = np.zeros((NLOC, F), np.float32)
            for j in range(NTILES):
                nslots = COLS[j] * 128
                gbuf = np.zeros((nslots, F), np.float32)
                als = np.zeros(nslots, np.float32)
                gs = im["gsi"][:16, GSOFF[j]:GSOFF[j + 1]]
                tA = np.arange(CA[j])
                idxA = gs[tA % 16, tA // 16].astype(np.int64)
                vA = idxA >= 0
                gbuf[tA[vA]] = table[idxA[vA]]
                als[tA[vA]] = als_tab[idxA[vA]]
                tB = np.arange(CB[j])
                idxB = gs[tB % 16, CA[j] // 16 + tB // 16].astype(np.int64)
                vB = idxB >= 0
                gbuf[128 * CACOLS[j] + tB[vB]] = table[BANK + idxB[vB]]
                als[128 * CACOLS[j] + tB[vB]] = als_tab[BANK + idxB[vB]]
                sg = im["segid"][:, OFF[j]:OFF[j + 1]]
                t = np.arange(nslots)
                seg_t = sg[t % 128, t // 128]
                ad_t = adt_all[t % 128, ICC * j + t // 128]
                e = als + ad_t
                e = np.maximum(e, NEG_SLOPE * e)
                p = np.exp(e).astype(np.float32)
                mask = seg_t[:, None] == np.arange(128)[None, :]
                sp = mask * p[:, None]                 # [slots, 128seg]
                out = sp.T @ gbuf                      # [128, F]
                ssum = sp.T @ np.ones(nslots, np.float32)
                h1 = out / (ssum[:, None] + EPS) + b[None, :]
                ho = np.where(h1 > 0, h1, np.exp(np.minimum(h1, 0)) - 1)
                ho_all[128 * j:128 * (j + 1)] = ho.astype(np.float32)
            if k < 2:
                new_hT.append(ho_all.T.copy())
            else:
                ph = im["phot"].reshape(NTILES, 128, G)
                for j in range(NTILES):
                    pool_part[r] += ph[j].T @ ho_all[128 * j:128 * (j + 1)]
        if k < 2:
            hT_cur = new_hT
    total = np.sum(pool_part, axis=0)
    return (total / np.maximum(counts, 1.0)[:, None]).astype(np.float32)


# ---------------- bass program ----------------

def _build_program(meta, repeat=1, nqueues=4, features=("gather", "ic", "cc", "mm")):
    import concourse.bacc as bacc
    import concourse.bass as bass
    import concourse.mybir as mybir
    import concourse.tile as tile
    from concourse import masks

    f32 = mybir.dt.float32
    bf16 = mybir.dt.bfloat16
    i16 = mybir.dt.int16
    u16 = mybir.dt.uint16
    AF = mybir.ActivationFunctionType
    ALU = mybir.AluOpType
    AX = mybir.AxisListType

    CA = [int(v) for v in meta["CA"]]
    CB = [int(v) for v in meta["CB"]]
    CACOLS = [int(v) for v in meta["CACOLS"]]
    COLS = [int(v) for v in meta["COLS"]]
    OFF = [int(v) for v in meta["OFF"]]
    GSOFF = [int(v) for v in meta["GSOFF"]]
    features = set(features)
    groups = meta["groups"]
    ICC = meta["ICC"]
    TOTCOLS = OFF[-1]
    GSTOT = GSOFF[-1]
    AW = sum(g[2] for g in groups)
    CMAX = max(COLS)
    ROWW = 256                # table row width (bf16): [Wh(128)|als|1|pad]
    TBLK = 3                  # table-build tiles per block (PSUM 3*132*4B)

    nc = bacc.Bacc("TRN2", target_bir_lowering=False, debug=False,
                   num_devices=NCORES, num_swdge_queues=nqueues)

    # --- dram I/O ---
    xT_own = nc.dram_tensor("xT_own", [F, NLOC], f32, kind="ExternalInput")
    w_aug_d = nc.dram_tensor("w_aug", [3, F, F + 2], f32, kind="ExternalInput")
    b_rep_d = nc.dram_tensor("b_rep", [3, 128, F], f32, kind="ExternalInput")
    gsi_d = nc.dram_tensor("gsi", [128, GSTOT], i16, kind="ExternalInput")
    segid_d = nc.dram_tensor("segid", [128, TOTCOLS], f32, kind="ExternalInput")
    aidx_d = nc.dram_tensor("aidx", [128, AW], u16, kind="ExternalInput")
    iota_d = nc.dram_tensor("iota", [128, 128], f32, kind="ExternalInput")
    phot_d = nc.dram_tensor("phot", [NTILES * 128, G], f32,
                            kind="ExternalInput")
    pool_out = nc.dram_tensor("pool_part", [G, F], f32, kind="ExternalOutput")

    # --- internal dram ---
    cc_in = nc.dram_tensor("cc_in", [NLOC, ROWW], bf16, kind="Internal")
    cc_out = nc.dram_tensor("cc_out", [NCORES, NLOC, ROWW], bf16,
                            kind="Internal", addr_space="Shared")
    table = cc_out.ap().rearrange("r n f -> (r n) f")

    with tile.TileContext(nc) as tc:
        with (
            tc.tile_pool(name="persist", bufs=1) as persist,
            tc.tile_pool(name="gb", bufs=4) as gb_pool,
            tc.tile_pool(name="sp", bufs=3) as sp_pool,
            tc.tile_pool(name="edge", bufs=3) as edge_pool,
            tc.tile_pool(name="post", bufs=3) as post_pool,
            tc.tile_pool(name="pst", bufs=2, space="PSUM") as pst_pool,
            tc.tile_pool(name="ptr", bufs=1, space="PSUM") as ptr_pool,
            tc.tile_pool(name="ps1", bufs=1, space="PSUM") as ps1_pool,
            tc.tile_pool(name="psb", bufs=1, space="PSUM") as psb_pool,
            tc.tile_pool(name="pso", bufs=2, space="PSUM") as pso_pool,
            tc.tile_pool(name="pspool", bufs=1, space="PSUM") as pspool_pool,
        ):
            # persistent tiles
            hT = persist.tile([F, NLOC], f32, tag="hT")
            ad_rep = persist.tile([128, NAUG], f32, tag="ad_rep")
            ad_row = persist.tile([1, NLOC], f32, tag="ad_row")
            adt_all = persist.tile([128, NTILES * ICC], f32, tag="adt_all")
            identity = persist.tile([128, 128], f32, tag="identity")
            ones_col = persist.tile([1, 128], f32, tag="ones_col")
            w_sb = persist.tile([F, F + 2], f32, tag="w_sb")
            b_sb = persist.tile([128, F], f32, tag="b_sb")
            ic_out = persist.tile([128, 512], f32, tag="ic_out")
            aidx_sb = persist.tile([128, AW], u16, tag="aidx_sb")
            gsi_sb = persist.tile([128, GSTOT], i16, tag="gsi_sb")
            segid_sb = persist.tile([128, TOTCOLS], f32, tag="segid_sb")
            iota_sb = persist.tile([128, 128], f32, tag="iota_sb")
            st_bufs = [persist.tile([128, TBLK, ROWW], bf16, tag=f"st{i}",
                                    name=f"st{i}") for i in range(3)]

            masks.make_identity(nc, identity[:])
            nc.gpsimd.memset(ones_col[:], 1.0)
            nc.sync.dma_start(aidx_sb[:], aidx_d.ap())
            nc.sync.dma_start(gsi_sb[:], gsi_d.ap())
            nc.sync.dma_start(segid_sb[:], segid_d.ap())
            nc.sync.dma_start(iota_sb[:], iota_d.ap())
            for stb in st_bufs:
                nc.vector.memset(stb[:], 0.0)
                nc.vector.memset(stb[:, :, F + 1:F + 2], 1.0)

            for rep in range(repeat):
              nc.sync.dma_start(hT[:], xT_own.ap())
              for k in range(3):
                  nc.sync.dma_start(w_sb[:], w_aug_d.ap()[k])
                  nc.sync.dma_start(b_sb[:], b_rep_d.ap()[k])

                  # ---- own-shard [Wh|als] -> cc_in (bf16); AllGather = table
                  pos = 0
                  blk = 0
                  while pos < NTILES:
                      nt = min(TBLK, NTILES - pos)
                      ps = pst_pool.tile([128, TBLK, 132], f32, tag="ps_tab")
                      for i in range(nt):
                          nc.tensor.matmul(
                              ps[:, i, 0:F + 1],
                              hT[:, 128 * (pos + i):128 * (pos + i + 1)],
                              w_sb[:, 0:F + 1], start=True, stop=True)
                      st = st_bufs[blk % 3]
                      nc.scalar.activation(st[:, 0:nt, 0:F + 1],
                                           ps[:, 0:nt, 0:F + 1], AF.Copy)
                      dst = cc_in.ap()[128 * pos:128 * (pos + nt)] \
                          .rearrange("(t p) f -> p t f", t=nt)
                      nc.scalar.dma_start(dst, st[:, 0:nt])
                      pos += nt
                      blk += 1
                  if "cc" in features:
                      nc.gpsimd.collective_compute(
                          "AllGather", mybir.AluOpType.bypass,
                          replica_groups=[list(range(NCORES))],
                          ins=[cc_in.ap().opt()], outs=[cc_out.ap().opt()])
                  else:
                      for rr in range(NCORES):
                          nc.sync.dma_start(cc_out.ap()[rr], cc_in.ap()[:])

                  # ---- alpha_d of own nodes -> replicated [128, NAUG] ----
                  ad_chunks = []
                  pos = 0
                  while pos < NLOC:
                      sz = min(512, NLOC - pos)
                      ad_chunks.append((pos, sz))
                      pos += sz
                  for pos, sz in ad_chunks:
                      sl = slice(pos, pos + sz)
                      pr = ps1_pool.tile([1, 512], f32, tag="ps_ad")
                      nc.tensor.matmul(pr[:, 0:sz], w_sb[:, F + 1:F + 2],
                                       hT[:, sl], start=True, stop=True)
                      nc.vector.tensor_copy(ad_row[:, sl], pr[:, 0:sz])
                  for pos, sz in ad_chunks:
                      sl = slice(pos, pos + sz)
                      pb = psb_pool.tile([128, 512], f32, tag="ps_bc")
                      nc.tensor.matmul(pb[:, 0:sz], ones_col[:], ad_row[:, sl],
                                       start=True, stop=True)
                      nc.vector.tensor_copy(ad_rep[:, sl], pb[:, 0:sz])
                  nc.vector.memset(ad_rep[:, NLOC:NAUG], BIG_NEG)

                  # ---- per-token alpha_d via grouped indirect copies ----
                  if "ic" in features:
                      aoff = 0
                      for (c0, gsz, C_all) in groups:
                          M = 16 * C_all
                          nc.gpsimd.indirect_copy(
                              ic_out[:, 0:M], ad_rep[:],
                              aidx_sb[:, aoff:aoff + C_all], True)
                          src_ap = ic_out[:, 0:M].rearrange(
                              "(g o) (kk j) -> g o kk j",
                              g=8, o=16, kk=16, j=C_all)[:, 0]
                          nc.sync.dma_start(
                              adt_all[:, ICC * c0:ICC * c0 + C_all], src_ap)
                          aoff += C_all
                  else:
                      nc.vector.memset(adt_all[:], 0.0)

                  # ---- edge phase: one chunk per dst tile ----
                  if k == 2:
                      ps_pl = pspool_pool.tile([G, F], f32, tag="ps_pl")
                  for j in range(NTILES):
                      cj = COLS[j]
                      gbuf = gb_pool.tile([128, CMAX, ROWW], bf16, tag="gbuf")
                      if "gather" not in features:
                          nc.vector.memset(gbuf[:], 0.0)
                      else:
                          nc.gpsimd.dma_gather(
                              gbuf[:, 0:CACOLS[j]], table[0:BANK],
                              gsi_sb[:, GSOFF[j]:GSOFF[j] + CA[j] // 16],
                              CA[j], CA[j], ROWW, single_packet=False,
                              queue_num=(2 * j) % nqueues)
                          nc.gpsimd.dma_gather(
                              gbuf[:, CACOLS[j]:cj], table[BANK:NPAD],
                              gsi_sb[:, GSOFF[j] + CA[j] // 16:GSOFF[j + 1]],
                              CB[j], CB[j], ROWW, single_packet=False,
                              queue_num=(2 * j + 1) % nqueues)

                      # e = als + alpha_d ; p = exp(leakyrelu(e))
                      e = edge_pool.tile([128, CMAX], f32, tag="e")
                      nc.vector.tensor_tensor(e[:, 0:cj], gbuf[:, 0:cj, F],
                                              adt_all[:, ICC * j:ICC * j + cj],
                                              ALU.add)
                      nc.vector.scalar_tensor_tensor(
                          e[:, 0:cj], e[:, 0:cj], NEG_SLOPE, e[:, 0:cj],
                          ALU.mult, ALU.max)
                      p_bf = edge_pool.tile([128, CMAX], bf16, tag="p_bf")
                      nc.scalar.activation(p_bf[:, 0:cj], e[:, 0:cj], AF.Exp)

                      # S'[token, seg] = p * (segid == seg)
                      sp = sp_pool.tile([128, CMAX, 128], bf16, tag="sp")
                      sg_b = segid_sb[:, OFF[j]:OFF[j + 1]] \
                          .unsqueeze(2).broadcast_to([128, cj, 128])
                      io_b = iota_sb[:].unsqueeze(1).broadcast_to([128, cj, 128])
                      nc.vector.tensor_tensor(sp[:, 0:cj], sg_b, io_b,
                                              ALU.is_equal)
                      p_b = p_bf[:, 0:cj].unsqueeze(2).broadcast_to([128, cj, 128])
                      nc.vector.tensor_tensor(sp[:, 0:cj], sp[:, 0:cj], p_b,
                                              ALU.mult)

                      # segment sums in PSUM: cols [Wh(128)|junk|sum_p]
                      ps_o = pso_pool.tile([128, 132], f32, tag="ps_o")
                      if "mm" in features:
                          for g in range(cj):
                              nc.tensor.matmul(ps_o[:, 0:F + 2], sp[:, g],
                                               gbuf[:, g, 0:F + 2],
                                               start=(g == 0),
                                               stop=(g == cj - 1))
                      else:
                          nc.tensor.matmul(ps_o[:, 0:F + 2], sp[:, 0],
                                           gbuf[:, 0, 0:F + 2],
                                           start=True, stop=True)

                      # ---- post: h = out/sum_p + b, ELU ----
                      s_t = post_pool.tile([128, 1], f32, tag="s_t")
                      nc.vector.tensor_scalar_add(s_t[:], ps_o[:, F + 1:F + 2],
                                                  EPS)
                      r_t = post_pool.tile([128, 1], f32, tag="r_t")
                      nc.vector.reciprocal(r_t[:], s_t[:])
                      h1 = post_pool.tile([128, F], f32, tag="h1")
                      nc.vector.tensor_scalar(h1[:], ps_o[:, 0:F], r_t[:], None,
                                              ALU.mult)
                      nc.vector.tensor_tensor(h1[:], h1[:], b_sb[:], ALU.add)
                      mn = post_pool.tile([128, F], f32, tag="mn")
                      nc.vector.tensor_scalar_min(mn[:], h1[:], 0.0)
                      ex = post_pool.tile([128, F], f32, tag="ex")
                      nc.scalar.activation(ex[:], mn[:], AF.Exp)
                      rl = post_pool.tile([128, F], f32, tag="rl")
                      nc.vector.tensor_scalar_max(rl[:], h1[:], 0.0)
                      ho = post_pool.tile([128, F], f32, tag="ho")
                      nc.vector.scalar_tensor_tensor(ho[:], ex[:], -1.0, rl[:],
                                                     ALU.add, ALU.add)
                      if k < 2:
                          pt = ptr_pool.tile([128, 128], f32, tag="ps_tr")
                          nc.tensor.transpose(pt[:], ho[:], identity[:])
                          nc.vector.tensor_copy(hT[:, 128 * j:128 * (j + 1)],
                                                pt[:])
                      else:
                          ph = post_pool.tile([128, G], f32, tag="ph")
                          nc.sync.dma_start(
                              ph[:], phot_d.ap()[128 * j:128 * (j + 1)])
                          nc.tensor.matmul(ps_pl[:], ph[:], ho[:],
                                           start=(j == 0),
                                           stop=(j == NTILES - 1))

                  if k == 2:
                      pl_sb = post_pool.tile([G, F], f32, tag="pl_sb")
                      nc.vector.tensor_copy(pl_sb[:], ps_pl[:])
                      nc.sync.dma_start(pool_out.ap()[:], pl_sb[:])

    nc.compile()
    return nc


# ---------------- entry point ----------------

LAST_EXEC_NS = None
LAST_META = None


def kernel(x, edge_index, batch,
           W1, a_src1, a_dst1, b1,
           W2, a_src2, a_dst2, b2,
           W3, a_src3, a_dst3, b3):
    global LAST_EXEC_NS, LAST_META
    x = np.asarray(x, np.float32)
    edge_index = np.asarray(edge_index)
    batch = np.asarray(batch)
    Ws = [np.asarray(W1, np.float32), np.asarray(W2, np.float32),
          np.asarray(W3, np.float32)]
    asrcs = [np.asarray(a_src1, np.float32), np.asarray(a_src2, np.float32),
             np.asarray(a_src3, np.float32)]
    adsts = [np.asarray(a_dst1, np.float32), np.asarray(a_dst2, np.float32),
             np.asarray(a_dst3, np.float32)]
    bs = [np.asarray(b1, np.float32), np.asarray(b2, np.float32),
          np.asarray(b3, np.float32)]

    in_maps, meta, counts = _prep_inputs(x, edge_index, batch, Ws, asrcs,
                                         adsts, bs)
    LAST_META = meta

    from concourse.bass_utils import run_bass_kernel_spmd
    nc = _build_program(meta)
    res = run_bass_kernel_spmd(nc, in_maps, core_ids=list(range(NCORES)))
    LAST_EXEC_NS = res.exec_time_ns
    total = np.zeros((G, F), np.float32)
    for r in range(NCORES):
        total += res.results[r]["pool_part"]
    out = total / np.maximum(counts, 1.0)[:, None]
    return out.astype(np.float32)


# revision 34
# speedup vs baseline: 1.0089x; 1.0089x over previous
"""GAT encoder (3-layer) on 8 Trainium2 NeuronCores — scatter-free design.

Sharding: nodes partitioned across cores (graph partition). Edges partitioned
by destination node; weights replicated.

Key design vs the earlier gather+scatter version: the HW profile showed the
GpSimd Q7 core serially generating DMA descriptors for dma_gather AND
dma_scatter_add (~30us per 2048-edge chunk) while the DMA engines idled at
~50%. This version removes the scatter entirely and shrinks the table build:

  1. Per layer, each core computes Wh for ITS OWN nodes only (49 matmuls)
     and the AllGather of those [NLOC, F] bf16 shards IS the gather table
     (node-major [NPAD, F] in shared DRAM). No redundant 392-tile build,
     no separate table store, bf16 rows halve gather bytes.
  2. Edges are grouped by destination TILE (128 consecutive dst nodes), one
     chunk per tile. Segment softmax + scatter-add happen ON-CHIP: a DVE
     iota-compare builds S'[token, seg] = p_token one-hot-weighted, and
     16 PE matmuls accumulate out[seg, :] += S'_g.T @ gbuf_g in PSUM
     (fp32 accumulate). Sum-of-p comes from S'_g.T @ ones. Post-processing
     (divide, bias, ELU, transpose/pool) runs per chunk from PSUM — the
     out_aug HBM round-trip is gone.
  3. Gather idx streams carry trailing -1 pads (trimmed by Q7 before
     descriptor gen, so per-core count variance costs nothing); in-stream
     pads are killed by segid=-1 (S' row = 0) and alpha_d sentinel -1e9.
  4. Gathers cycle over 4 SWDGE queues so one chunk's drain overlaps the
     next chunk's descriptor generation.
"""

import math
import numpy as np

# ---------------- constants (hardcoded problem shape) ----------------
N = 50000
F = 128
G = 64
NCORES = 8
NLOC = 6272                   # 49*128 nodes per core (padded)
NPAD = NLOC * NCORES          # 50176
NTILES = NLOC // 128          # 49 = dst tiles per core = chunks per layer
BANK = 32768                  # gather bank split (int16 idx range)
NAUG = NLOC + 64              # alpha_d replicated width (sentinel tail)
NEG_SLOPE = 0.2
BIG_NEG = -1.0e9
EPS = 1.0e-16
IC_GROUP = 2                  # chunks per indirect-copy call (ISA dst limit 512)
PADFILL = True                # pad gather idx streams with 0 to full width


# ---------------- host-side preprocessing ----------------

def _build_edge_data(src, dst):
    """Group edges by (core, dst-tile, src-bank); build per-chunk gather idx
    streams (wrapped-16, trailing -1 pads), segid arrays, and the alpha_d
    indirect-copy idx stream."""
    per = {}
    for r in range(NCORES):
        lo, hi = r * NLOC, (r + 1) * NLOC
        m = (dst >= lo) & (dst < hi)
        gs = src[m].astype(np.int64)
        ld = (dst[m] - lo).astype(np.int64)
        tile = ld // 128
        seg = ld % 128
        bankB = gs >= BANK
        for j in range(NTILES):
            tm = tile == j
            mA = tm & ~bankB
            mB = tm & bankB
            per[(r, j)] = (gs[mA], seg[mA], gs[mB] - BANK, seg[mB])

    # per-chunk bank widths (max over cores, rounded to 16)
    CA = np.zeros(NTILES, np.int64)
    CB = np.zeros(NTILES, np.int64)
    for j in range(NTILES):
        for r in range(NCORES):
            gA, _, gB, _ = per[(r, j)]
            CA[j] = max(CA[j], len(gA))
            CB[j] = max(CB[j], len(gB))
    CA = ((CA + 15) // 16) * 16
    CB = ((CB + 15) // 16) * 16
    CACOLS = (CA + 127) // 128
    CBCOLS = (CB + 127) // 128
    if PADFILL:
        CA = CACOLS * 128
        CB = CBCOLS * 128
    COLS = CACOLS + CBCOLS                      # slot cols per chunk
    OFF = np.concatenate([[0], np.cumsum(COLS)])  # cumulative col offsets
    GSW = (CA + CB) // 16                        # idx words per chunk
    GSOFF = np.concatenate([[0], np.cumsum(GSW)])

    def wrap16(vals, width):
        """vals (int) -> [16, width//16] wrapped (token t -> [t%16, t//16]),
        then tiled to [128, width//16]."""
        a = np.full(width, 0 if PADFILL else -1, np.int64)
        a[:len(vals)] = vals
        t = np.arange(width)
        w = np.zeros((16, width // 16), np.int16)
        w[t % 16, t // 16] = a.astype(np.int16)
        return np.tile(w, (8, 1))

    # adt IC stream is padded to a fixed ICC=16 cols (2048 slots) per chunk
    # so the grouped indirect copies keep the known-good 512-elem geometry.
    ICC = 16
    assert COLS.max() <= ICC
    gsi = np.zeros((NCORES, 128, GSOFF[-1]), np.int16)
    segid = np.full((NCORES, 128, OFF[-1]), -1.0, np.float32)
    ld_tok = np.full((NCORES, NTILES * ICC * 128), NLOC, np.int64)

    for r in range(NCORES):
        for j in range(NTILES):
            gA, sA, gB, sB = per[(r, j)]
            gsi[r, :, GSOFF[j]:GSOFF[j] + CA[j] // 16] = wrap16(gA, CA[j])
            gsi[r, :, GSOFF[j] + CA[j] // 16:GSOFF[j + 1]] = wrap16(gB, CB[j])
            # slot s (= col*128 + p) -> seg / local dst
            base = j * ICC * 128
            for (g, s, s0) in ((gA, sA, 0), (gB, sB, 128 * CACOLS[j])):
                tt = s0 + np.arange(len(g))
                segid[r, tt % 128, OFF[j] + tt // 128] = s.astype(np.float32)
                ld_tok[r, base + tt] = j * 128 + s
    return dict(per=per, CA=CA, CB=CB, CACOLS=CACOLS, CBCOLS=CBCOLS,
                COLS=COLS, OFF=OFF, GSW=GSW, GSOFF=GSOFF,
                ICC=ICC), gsi, segid, ld_tok


def _ic_groups(n_chunks, icc, ic_limit=512):
    """Group consecutive chunks for the alpha_d indirect copy; each group
    covers gsz chunks of icc cols each, 16*gsz*icc <= ic_limit."""
    per = ic_limit // (16 * icc)
    groups = []
    pos = 0
    while pos < n_chunks:
        sz = min(per, n_chunks - pos)
        groups.append((pos, sz, sz * icc))
        pos += sz
    return groups


def _build_aidx(ld_tok_r, meta, groups):
    """Build the u16 idx stream for the grouped indirect copies.
    Group covering chunks [c0, c0+gsz) has C_all = gsz*ICC columns;
    IC output stream position i on partition 16*gg + (i%16), col i//16 maps
    to token tt = j*128 + 16*gg + k where k = i//C_all, j = i%C_all
    (each 16-partition group gg covers tokens with residue [16gg, 16gg+16))."""
    icc = meta["ICC"]
    parts = []
    for (c0, gsz, C_all) in groups:
        M = 16 * C_all
        out = np.zeros((128, M // 16), np.uint16)
        i_arr = np.arange(M)
        k_arr = i_arr // C_all
        j_arr = i_arr % C_all
        base = c0 * icc * 128
        ld = ld_tok_r[base:base + 128 * C_all]
        for gg in range(8):
            tt = j_arr * 128 + 16 * gg + k_arr
            out[16 * gg + i_arr % 16, i_arr // 16] = ld[tt].astype(np.uint16)
        parts.append(out)
    return np.concatenate(parts, axis=1)


def _prep_inputs(x, edge_index, batch, Ws, asrcs, adsts, bs):
    src = np.concatenate([edge_index[0], np.arange(N, dtype=np.int64)])
    dst = np.concatenate([edge_index[1], np.arange(N, dtype=np.int64)])
    src = np.asarray(src, np.int64)
    dst = np.asarray(dst, np.int64)

    meta, gsi, segid, ld_tok = _build_edge_data(src, dst)
    groups = _ic_groups(NTILES, meta["ICC"])
    meta["groups"] = groups

    xT_own = np.zeros((NCORES, F, NLOC), np.float32)
    xf = np.asarray(x, np.float32).T
    for r in range(NCORES):
        lo = r * NLOC
        w = min(NLOC, max(0, N - lo))
        xT_own[r, :, :w] = xf[:, lo:lo + w]

    w_aug = np.zeros((3, F, F + 2), np.float32)
    for k in range(3):
        w_aug[k, :, :F] = Ws[k]
        w_aug[k, :, F] = Ws[k] @ asrcs[k]
        w_aug[k, :, F + 1] = Ws[k] @ adsts[k]

    b_rep = np.zeros((3, 128, F), np.float32)
    for k in range(3):
        b_rep[k] = np.tile(bs[k][None, :], (128, 1))

    iota = np.tile(np.arange(128, dtype=np.float32)[None, :], (128, 1))

    batch64 = np.asarray(batch, np.int64)
    phot = np.zeros((NCORES, NTILES, 128, G), np.float32)
    for r in range(NCORES):
        base = r * NLOC
        for j in range(NTILES):
            nodes = base + j * 128 + np.arange(128)
            valid = nodes < N
            gsel = batch64[np.minimum(nodes, N - 1)]
            ph = np.zeros((128, G), np.float32)
            ph[np.arange(128)[valid], gsel[valid]] = 1.0
            phot[r, j] = ph

    counts = np.bincount(batch64, minlength=G).astype(np.float32)

    in_maps = []
    for r in range(NCORES):
        in_maps.append({
            "xT_own": np.ascontiguousarray(xT_own[r]),
            "w_aug": w_aug,
            "b_rep": b_rep,
            "gsi": np.ascontiguousarray(gsi[r]),
            "segid": np.ascontiguousarray(segid[r]),
            "aidx": _build_aidx(ld_tok[r], meta, groups),
            "iota": iota,
            "phot": phot[r].reshape(NTILES * 128, G),
        })
    return in_maps, meta, counts


# ---------------- numpy emulation of the device program ----------------

def _emulate_full(in_maps, meta, counts):
    CA, CB = meta["CA"], meta["CB"]
    CACOLS, COLS, OFF, GSOFF = meta["CACOLS"], meta["COLS"], meta["OFF"], meta["GSOFF"]
    hT_cur = [im["xT_own"].copy() for im in in_maps]
    pool_part = [np.zeros((G, F), np.float32) for _ in range(NCORES)]
    for k in range(3):
        # table = allgather of own Wh
        tabs = []
        alss = []
        ad_reps = []
        for r in range(NCORES):
            w = in_maps[r]["w_aug"][k]
            tabs.append((hT_cur[r].T @ w[:, :F]).astype(np.float32))
            alss.append((hT_cur[r].T @ w[:, F]).astype(np.float32))
            ad = (w[:, F + 1][None, :] @ hT_cur[r])[0]
            ad_aug = np.full(NAUG, BIG_NEG, np.float32)
            ad_aug[:NLOC] = ad
            ad_reps.append(ad_aug)
        table = np.concatenate(tabs, axis=0)          # [NPAD, F]
        als_tab = np.concatenate(alss, axis=0)        # [NPAD]
        new_hT = []
        for r in range(NCORES):
            im = in_maps[r]
            b = im["b_rep"][k][0]
            # adt via the aidx emulation (validates _build_aidx)
            ICC = meta["ICC"]
            adt_all = np.zeros((128, NTILES * ICC), np.float32)
            aoff = 0
            for (c0, gsz, C_all) in meta["groups"]:
                M = 16 * C_all
                i_arr = np.arange(M)
                k_arr = i_arr // C_all
                j_arr = i_arr % C_all
                a16 = im["aidx"][:, aoff:aoff + C_all].astype(np.int64)
                aoff += C_all
                for gg in range(8):
                    stream = a16[16 * gg + i_arr % 16, i_arr // 16]
                    vals = ad_reps[r][stream]
                    # stream pos i -> token (j*128 + 16gg + k) of group slots
                    tt = j_arr * 128 + 16 * gg + k_arr
                    adt_all[tt % 128, ICC * c0 + tt // 128] = vals
            ho_all = np.zeros((NLOC, F), np.float32)
            for j in range(NTILES):
                nslots = COLS[j] * 128
                gbuf = np.zeros((nslots, F), np.float32)
                als = np.zeros(nslots, np.float32)
                gs = im["gsi"][:16, GSOFF[j]:GSOFF[j + 1]]
                tA = np.arange(CA[j])
                idxA = gs[tA % 16, tA // 16].astype(np.int64)
                vA = idxA >= 0
                gbuf[tA[vA]] = table[idxA[vA]]
                als[tA[vA]] = als_tab[idxA[vA]]
                tB = np.arange(CB[j])
                idxB = gs[tB % 16, CA[j] // 16 + tB // 16].astype(np.int64)
                vB = idxB >= 0
                gbuf[128 * CACOLS[j] + tB[vB]] = table[BANK + idxB[vB]]
                als[128 * CACOLS[j] + tB[vB]] = als_tab[BANK + idxB[vB]]
                sg = im["segid"][:, OFF[j]:OFF[j + 1]]
                t = np.arange(nslots)
                seg_t = sg[t % 128, t // 128]
                ad_t = adt_all[t % 128, ICC * j + t // 128]
                e = als + ad_t
                e = np.maximum(e, NEG_SLOPE * e)
                p = np.exp(e).astype(np.float32)
                mask = seg_t[:, None] == np.arange(128)[None, :]
                sp = mask * p[:, None]                 # [slots, 128seg]
                out = sp.T @ gbuf                      # [128, F]
                ssum = sp.T @ np.ones(nslots, np.float32)
                h1 = out / (ssum[:, None] + EPS) + b[None, :]
                ho = np.where(h1 > 0, h1, np.exp(np.minimum(h1, 0)) - 1)
                ho_all[128 * j:128 * (j + 1)] = ho.astype(np.float32)
            if k < 2:
                new_hT.append(ho_all.T.copy())
            else:
                ph = im["phot"].reshape(NTILES, 128, G)
                for j in range(NTILES):
                    pool_part[r] += ph[j].T @ ho_all[128 * j:128 * (j + 1)]
        if k < 2:
            hT_cur = new_hT
    total = np.sum(pool_part, axis=0)
    return (total / np.maximum(counts, 1.0)[:, None]).astype(np.float32)


# ---------------- bass program ----------------

def _build_program(meta, repeat=1, nqueues=4, features=("gather", "ic", "cc", "mm")):
    import concourse.bacc as bacc
    import concourse.bass as bass
    import concourse.mybir as mybir
    import concourse.tile as tile
    from concourse import masks

    f32 = mybir.dt.float32
    bf16 = mybir.dt.bfloat16
    i16 = mybir.dt.int16
    u16 = mybir.dt.uint16
    AF = mybir.ActivationFunctionType
    ALU = mybir.AluOpType
    AX = mybir.AxisListType

    CA = [int(v) for v in meta["CA"]]
    CB = [int(v) for v in meta["CB"]]
    CACOLS = [int(v) for v in meta["CACOLS"]]
    COLS = [int(v) for v in meta["COLS"]]
    OFF = [int(v) for v in meta["OFF"]]
    GSOFF = [int(v) for v in meta["GSOFF"]]
    features = set(features)
    groups = meta["groups"]
    ICC = meta["ICC"]
    TOTCOLS = OFF[-1]
    GSTOT = GSOFF[-1]
    AW = sum(g[2] for g in groups)
    CMAX = max(COLS)
    ROWW = 256                # table row width (bf16): [Wh(128)|als|1|pad]
    TBLK = 3                  # table-build tiles per block (PSUM 3*132*4B)

    nc = bacc.Bacc("TRN2", target_bir_lowering=False, debug=False,
                   num_devices=NCORES, num_swdge_queues=nqueues)

    # --- dram I/O ---
    xT_own = nc.dram_tensor("xT_own", [F, NLOC], f32, kind="ExternalInput")
    w_aug_d = nc.dram_tensor("w_aug", [3, F, F + 2], f32, kind="ExternalInput")
    b_rep_d = nc.dram_tensor("b_rep", [3, 128, F], f32, kind="ExternalInput")
    gsi_d = nc.dram_tensor("gsi", [128, GSTOT], i16, kind="ExternalInput")
    segid_d = nc.dram_tensor("segid", [128, TOTCOLS], f32, kind="ExternalInput")
    aidx_d = nc.dram_tensor("aidx", [128, AW], u16, kind="ExternalInput")
    iota_d = nc.dram_tensor("iota", [128, 128], f32, kind="ExternalInput")
    phot_d = nc.dram_tensor("phot", [NTILES * 128, G], f32,
                            kind="ExternalInput")
    pool_out = nc.dram_tensor("pool_part", [G, F], f32, kind="ExternalOutput")

    # --- internal dram ---
    cc_in = nc.dram_tensor("cc_in", [NLOC, ROWW], bf16, kind="Internal")
    cc_out = nc.dram_tensor("cc_out", [NCORES, NLOC, ROWW], bf16,
                            kind="Internal", addr_space="Shared")
    table = cc_out.ap().rearrange("r n f -> (r n) f")

    with tile.TileContext(nc) as tc:
        with (
            tc.tile_pool(name="persist", bufs=1) as persist,
            tc.tile_pool(name="gb", bufs=4) as gb_pool,
            tc.tile_pool(name="sp", bufs=3) as sp_pool,
            tc.tile_pool(name="edge", bufs=3) as edge_pool,
            tc.tile_pool(name="post", bufs=3) as post_pool,
            tc.tile_pool(name="pst", bufs=2, space="PSUM") as pst_pool,
            tc.tile_pool(name="ptr", bufs=1, space="PSUM") as ptr_pool,
            tc.tile_pool(name="ps1", bufs=1, space="PSUM") as ps1_pool,
            tc.tile_pool(name="psb", bufs=1, space="PSUM") as psb_pool,
            tc.tile_pool(name="pso", bufs=2, space="PSUM") as pso_pool,
            tc.tile_pool(name="pspool", bufs=1, space="PSUM") as pspool_pool,
        ):
            # persistent tiles
            hT = persist.tile([F, NLOC], f32, tag="hT")
            ad_rep = persist.tile([128, NAUG], f32, tag="ad_rep")
            ad_row = persist.tile([1, NLOC], f32, tag="ad_row")
            adt_all = persist.tile([128, NTILES * ICC], f32, tag="adt_all")
            identity = persist.tile([128, 128], f32, tag="identity")
            ones_col = persist.tile([1, 128], f32, tag="ones_col")
            w_sb = persist.tile([F, F + 2], f32, tag="w_sb")
            b_sb = persist.tile([128, F], f32, tag="b_sb")
            ic_out = persist.tile([128, 512], f32, tag="ic_out")
            aidx_sb = persist.tile([128, AW], u16, tag="aidx_sb")
            gsi_sb = persist.tile([128, GSTOT], i16, tag="gsi_sb")
            segid_sb = persist.tile([128, TOTCOLS], f32, tag="segid_sb")
            iota_sb = persist.tile([128, 128], f32, tag="iota_sb")
            st_bufs = [persist.tile([128, TBLK, ROWW], bf16, tag=f"st{i}",
                                    name=f"st{i}") for i in range(3)]

            masks.make_identity(nc, identity[:])
            nc.gpsimd.memset(ones_col[:], 1.0)
            nc.sync.dma_start(aidx_sb[:], aidx_d.ap())
            nc.sync.dma_start(gsi_sb[:], gsi_d.ap())
            nc.sync.dma_start(segid_sb[:], segid_d.ap())
            nc.sync.dma_start(iota_sb[:], iota_d.ap())
            for stb in st_bufs:
                nc.vector.memset(stb[:], 0.0)
                nc.vector.memset(stb[:, :, F + 1:F + 2], 1.0)

            for rep in range(repeat):
              nc.sync.dma_start(hT[:], xT_own.ap())
              for k in range(3):
                  nc.sync.dma_start(w_sb[:], w_aug_d.ap()[k])
                  nc.sync.dma_start(b_sb[:], b_rep_d.ap()[k])

                  # ---- own-shard [Wh|als] -> cc_in (bf16); AllGather = table
                  pos = 0
                  blk = 0
                  while pos < NTILES:
                      nt = min(TBLK, NTILES - pos)
                      ps = pst_pool.tile([128, TBLK, 132], f32, tag="ps_tab")
                      for i in range(nt):
                          nc.tensor.matmul(
                              ps[:, i, 0:F + 1],
                              hT[:, 128 * (pos + i):128 * (pos + i + 1)],
                              w_sb[:, 0:F + 1], start=True, stop=True)
                      st = st_bufs[blk % 3]
                      nc.scalar.activation(st[:, 0:nt, 0:F + 1],
                                           ps[:, 0:nt, 0:F + 1], AF.Copy)
                      dst = cc_in.ap()[128 * pos:128 * (pos + nt)] \
                          .rearrange("(t p) f -> p t f", t=nt)
                      nc.scalar.dma_start(dst, st[:, 0:nt])
                      pos += nt
                      blk += 1
                  if "cc" in features:
                      nc.gpsimd.collective_compute(
                          "AllGather", mybir.AluOpType.bypass,
                          replica_groups=[list(range(NCORES))],
                          ins=[cc_in.ap().opt()], outs=[cc_out.ap().opt()])
                  else:
                      for rr in range(NCORES):
                          nc.sync.dma_start(cc_out.ap()[rr], cc_in.ap()[:])

                  # ---- alpha_d of own nodes -> replicated [128, NAUG] ----
                  ad_chunks = []
                  pos = 0
                  while pos < NLOC:
                      sz = min(512, NLOC - pos)
                      ad_chunks.append((pos, sz))
                      pos += sz
                  for pos, sz in ad_chunks:
                      sl = slice(pos, pos + sz)
                      pr = ps1_pool.tile([1, 512], f32, tag="ps_ad")
                      nc.tensor.matmul(pr[:, 0:sz], w_sb[:, F + 1:F + 2],
                                       hT[:, sl], start=True, stop=True)
                      nc.vector.tensor_copy(ad_row[:, sl], pr[:, 0:sz])
                  for pos, sz in ad_chunks:
                      sl = slice(pos, pos + sz)
                      pb = psb_pool.tile([128, 512], f32, tag="ps_bc")
                      nc.tensor.matmul(pb[:, 0:sz], ones_col[:], ad_row[:, sl],
                                       start=True, stop=True)
                      nc.vector.tensor_copy(ad_rep[:, sl], pb[:, 0:sz])
                  nc.vector.memset(ad_rep[:, NLOC:NAUG], BIG_NEG)

                  # ---- per-token alpha_d via grouped indirect copies ----
                  if "ic" in features:
                      aoff = 0
                      for (c0, gsz, C_all) in groups:
                          M = 16 * C_all
                          nc.gpsimd.indirect_copy(
                              ic_out[:, 0:M], ad_rep[:],
                              aidx_sb[:, aoff:aoff + C_all], True)
                          src_ap = ic_out[:, 0:M].rearrange(
                              "(g o) (kk j) -> g o kk j",
                              g=8, o=16, kk=16, j=C_all)[:, 0]
                          nc.sync.dma_start(
                              adt_all[:, ICC * c0:ICC * c0 + C_all], src_ap)
                          aoff += C_all
                  else:
                      nc.vector.memset(adt_all[:], 0.0)

                  # ---- edge phase: one chunk per dst tile ----
                  # Software-pipelined: chunk j's post-processing is emitted
                  # after chunk j+1's mask/matmuls so DVE never stalls on PE.
                  if k == 2:
                      ps_pl = pspool_pool.tile([G, F], f32, tag="ps_pl")

                  def emit_front(j):
                      cj = COLS[j]
                      gbuf = gb_pool.tile([128, CMAX, ROWW], bf16, tag="gbuf")
                      if "gather" not in features:
                          nc.vector.memset(gbuf[:], 0.0)
                      else:
                          nc.gpsimd.dma_gather(
                              gbuf[:, 0:CACOLS[j]], table[0:BANK],
                              gsi_sb[:, GSOFF[j]:GSOFF[j] + CA[j] // 16],
                              CA[j], CA[j], ROWW, single_packet=False,
                              queue_num=(2 * j) % nqueues)
                          nc.gpsimd.dma_gather(
                              gbuf[:, CACOLS[j]:cj], table[BANK:NPAD],
                              gsi_sb[:, GSOFF[j] + CA[j] // 16:GSOFF[j + 1]],
                              CB[j], CB[j], ROWW, single_packet=False,
                              queue_num=(2 * j + 1) % nqueues)

                      # e = als + alpha_d ; p = exp(leakyrelu(e))
                      e = edge_pool.tile([128, CMAX], f32, tag="e")
                      nc.vector.tensor_tensor(e[:, 0:cj], gbuf[:, 0:cj, F],
                                              adt_all[:, ICC * j:ICC * j + cj],
                                              ALU.add)
                      nc.vector.scalar_tensor_tensor(
                          e[:, 0:cj], e[:, 0:cj], NEG_SLOPE, e[:, 0:cj],
                          ALU.mult, ALU.max)
                      p_bf = edge_pool.tile([128, CMAX], bf16, tag="p_bf")
                      nc.scalar.activation(p_bf[:, 0:cj], e[:, 0:cj], AF.Exp)

                      # S'[token, seg] = p * (segid == seg)
                      sp = sp_pool.tile([128, CMAX, 128], bf16, tag="sp")
                      sg_b = segid_sb[:, OFF[j]:OFF[j + 1]] \
                          .unsqueeze(2).broadcast_to([128, cj, 128])
                      io_b = iota_sb[:].unsqueeze(1).broadcast_to([128, cj, 128])
                      nc.vector.tensor_tensor(sp[:, 0:cj], sg_b, io_b,
                                              ALU.is_equal)
                      p_b = p_bf[:, 0:cj].unsqueeze(2).broadcast_to([128, cj, 128])
                      nc.vector.tensor_tensor(sp[:, 0:cj], sp[:, 0:cj], p_b,
                                              ALU.mult)

                      # segment sums in PSUM: cols [Wh(128)|junk|sum_p]
                      ps_o = pso_pool.tile([128, 132], f32, tag="ps_o")
                      if "mm" in features:
                          for g in range(cj):
                              nc.tensor.matmul(ps_o[:, 0:F + 2], sp[:, g],
                                               gbuf[:, g, 0:F + 2],
                                               start=(g == 0),
                                               stop=(g == cj - 1))
                      else:
                          nc.tensor.matmul(ps_o[:, 0:F + 2], sp[:, 0],
                                           gbuf[:, 0, 0:F + 2],
                                           start=True, stop=True)
                      return ps_o

                  def emit_post(j, ps_o):
                      s_t = post_pool.tile([128, 1], f32, tag="s_t")
                      nc.vector.tensor_scalar_add(s_t[:], ps_o[:, F + 1:F + 2],
                                                  EPS)
                      r_t = post_pool.tile([128, 1], f32, tag="r_t")
                      nc.vector.reciprocal(r_t[:], s_t[:])
                      h1 = post_pool.tile([128, F], f32, tag="h1")
                      nc.vector.tensor_scalar(h1[:], ps_o[:, 0:F], r_t[:], None,
                                              ALU.mult)
                      nc.vector.tensor_tensor(h1[:], h1[:], b_sb[:], ALU.add)
                      mn = post_pool.tile([128, F], f32, tag="mn")
                      nc.vector.tensor_scalar_min(mn[:], h1[:], 0.0)
                      ex = post_pool.tile([128, F], f32, tag="ex")
                      nc.scalar.activation(ex[:], mn[:], AF.Exp)
                      rl = post_pool.tile([128, F], f32, tag="rl")
                      nc.vector.tensor_scalar_max(rl[:], h1[:], 0.0)
                      ho = post_pool.tile([128, F], f32, tag="ho")
                      nc.vector.scalar_tensor_tensor(ho[:], ex[:], -1.0, rl[:],
                                                     ALU.add, ALU.add)
                      if k < 2:
                          pt = ptr_pool.tile([128, 128], f32, tag="ps_tr")
                          nc.tensor.transpose(pt[:], ho[:], identity[:])
                          nc.scalar.activation(hT[:, 128 * j:128 * (j + 1)],
                                               pt[:], AF.Copy)
                      else:
                          ph = post_pool.tile([128, G], f32, tag="ph")
                          nc.sync.dma_start(
                              ph[:], phot_d.ap()[128 * j:128 * (j + 1)])
                          nc.tensor.matmul(ps_pl[:], ph[:], ho[:],
                                           start=(j == 0),
                                           stop=(j == NTILES - 1))

                  pending = None
                  for j in range(NTILES):
                      ps_o = emit_front(j)
                      if pending is not None:
                          emit_post(pending[0], pending[1])
                      pending = (j, ps_o)
                  emit_post(pending[0], pending[1])

                  if k == 2:
                      pl_sb = post_pool.tile([G, F], f32, tag="pl_sb")
                      nc.vector.tensor_copy(pl_sb[:], ps_pl[:])
                      nc.sync.dma_start(pool_out.ap()[:], pl_sb[:])

    nc.compile()
    return nc


# ---------------- entry point ----------------

LAST_EXEC_NS = None
LAST_META = None


def kernel(x, edge_index, batch,
           W1, a_src1, a_dst1, b1,
           W2, a_src2, a_dst2, b2,
           W3, a_src3, a_dst3, b3):
    global LAST_EXEC_NS, LAST_META
    x = np.asarray(x, np.float32)
    edge_index = np.asarray(edge_index)
    batch = np.asarray(batch)
    Ws = [np.asarray(W1, np.float32), np.asarray(W2, np.float32),
          np.asarray(W3, np.float32)]
    asrcs = [np.asarray(a_src1, np.float32), np.asarray(a_src2, np.float32),
             np.asarray(a_src3, np.float32)]
    adsts = [np.asarray(a_dst1, np.float32), np.asarray(a_dst2, np.float32),
             np.asarray(a_dst3, np.float32)]
    bs = [np.asarray(b1, np.float32), np.asarray(b2, np.float32),
          np.asarray(b3, np.float32)]

    in_maps, meta, counts = _prep_inputs(x, edge_index, batch, Ws, asrcs,
                                         adsts, bs)
    LAST_META = meta

    from concourse.bass_utils import run_bass_kernel_spmd
    nc = _build_program(meta)
    res = run_bass_kernel_spmd(nc, in_maps, core_ids=list(range(NCORES)))
    LAST_EXEC_NS = res.exec_time_ns
    total = np.zeros((G, F), np.float32)
    for r in range(NCORES):
        total += res.results[r]["pool_part"]
    out = total / np.maximum(counts, 1.0)[:, None]
    return out.astype(np.float32)


# revision 36
# speedup vs baseline: 1.0353x; 1.0262x over previous
"""GAT encoder (3-layer) on 8 Trainium2 NeuronCores — scatter-free design.

Sharding: nodes partitioned across cores (graph partition). Edges partitioned
by destination node; weights replicated.

Key design vs the earlier gather+scatter version: the HW profile showed the
GpSimd Q7 core serially generating DMA descriptors for dma_gather AND
dma_scatter_add (~30us per 2048-edge chunk) while the DMA engines idled at
~50%. This version removes the scatter entirely and shrinks the table build:

  1. Per layer, each core computes Wh for ITS OWN nodes only (49 matmuls)
     and the AllGather of those [NLOC, F] bf16 shards IS the gather table
     (node-major [NPAD, F] in shared DRAM). No redundant 392-tile build,
     no separate table store, bf16 rows halve gather bytes.
  2. Edges are grouped by destination TILE (128 consecutive dst nodes), one
     chunk per tile. Segment softmax + scatter-add happen ON-CHIP: a DVE
     iota-compare builds S'[token, seg] = p_token one-hot-weighted, and
     16 PE matmuls accumulate out[seg, :] += S'_g.T @ gbuf_g in PSUM
     (fp32 accumulate). Sum-of-p comes from S'_g.T @ ones. Post-processing
     (divide, bias, ELU, transpose/pool) runs per chunk from PSUM — the
     out_aug HBM round-trip is gone.
  3. Gather idx streams carry trailing -1 pads (trimmed by Q7 before
     descriptor gen, so per-core count variance costs nothing); in-stream
     pads are killed by segid=-1 (S' row = 0) and alpha_d sentinel -1e9.
  4. Gathers cycle over 4 SWDGE queues so one chunk's drain overlaps the
     next chunk's descriptor generation.
"""

import math
import numpy as np

# ---------------- constants (hardcoded problem shape) ----------------
N = 50000
F = 128
G = 64
NCORES = 8
NLOC = 6272                   # 49*128 nodes per core (padded)
NPAD = NLOC * NCORES          # 50176
NTILES = NLOC // 128          # 49 = dst tiles per core = chunks per layer
BANK = 32768                  # gather bank split (int16 idx range)
NAUG = NLOC + 64              # alpha_d replicated width (sentinel tail)
NEG_SLOPE = 0.2
BIG_NEG = -1.0e9
EPS = 1.0e-16
IC_GROUP = 2                  # chunks per indirect-copy call (ISA dst limit 512)
PADFILL = True                # pad gather idx streams with 0 to full width


# ---------------- host-side preprocessing ----------------

def _build_edge_data(src, dst):
    """Group edges by (core, dst-tile, src-bank); build per-chunk gather idx
    streams (wrapped-16, trailing -1 pads), segid arrays, and the alpha_d
    indirect-copy idx stream."""
    per = {}
    for r in range(NCORES):
        lo, hi = r * NLOC, (r + 1) * NLOC
        m = (dst >= lo) & (dst < hi)
        gs = src[m].astype(np.int64)
        ld = (dst[m] - lo).astype(np.int64)
        tile = ld // 128
        seg = ld % 128
        bankB = gs >= BANK
        for j in range(NTILES):
            tm = tile == j
            mA = tm & ~bankB
            mB = tm & bankB
            per[(r, j)] = (gs[mA], seg[mA], gs[mB] - BANK, seg[mB])

    # per-chunk bank widths (max over cores, rounded to 16)
    CA = np.zeros(NTILES, np.int64)
    CB = np.zeros(NTILES, np.int64)
    for j in range(NTILES):
        for r in range(NCORES):
            gA, _, gB, _ = per[(r, j)]
            CA[j] = max(CA[j], len(gA))
            CB[j] = max(CB[j], len(gB))
    CA = ((CA + 15) // 16) * 16
    CB = ((CB + 15) // 16) * 16
    CACOLS = (CA + 127) // 128
    CBCOLS = (CB + 127) // 128
    if PADFILL:
        CA = CACOLS * 128
        CB = CBCOLS * 128
    COLS = CACOLS + CBCOLS                      # slot cols per chunk
    OFF = np.concatenate([[0], np.cumsum(COLS)])  # cumulative col offsets
    GSW = (CA + CB) // 16                        # idx words per chunk
    GSOFF = np.concatenate([[0], np.cumsum(GSW)])

    def wrap16(vals, width):
        """vals (int) -> [16, width//16] wrapped (token t -> [t%16, t//16]),
        then tiled to [128, width//16]."""
        a = np.full(width, 0 if PADFILL else -1, np.int64)
        a[:len(vals)] = vals
        t = np.arange(width)
        w = np.zeros((16, width // 16), np.int16)
        w[t % 16, t // 16] = a.astype(np.int16)
        return np.tile(w, (8, 1))

    # adt IC stream is padded to a fixed ICC=16 cols (2048 slots) per chunk
    # so the grouped indirect copies keep the known-good 512-elem geometry.
    ICC = 16
    assert COLS.max() <= ICC
    gsi = np.zeros((NCORES, 128, GSOFF[-1]), np.int16)
    segid = np.full((NCORES, 128, OFF[-1]), -1.0, np.float32)
    ld_tok = np.full((NCORES, NTILES * ICC * 128), NLOC, np.int64)

    for r in range(NCORES):
        for j in range(NTILES):
            gA, sA, gB, sB = per[(r, j)]
            gsi[r, :, GSOFF[j]:GSOFF[j] + CA[j] // 16] = wrap16(gA, CA[j])
            gsi[r, :, GSOFF[j] + CA[j] // 16:GSOFF[j + 1]] = wrap16(gB, CB[j])
            # slot s (= col*128 + p) -> seg / local dst
            base = j * ICC * 128
            for (g, s, s0) in ((gA, sA, 0), (gB, sB, 128 * CACOLS[j])):
                tt = s0 + np.arange(len(g))
                segid[r, tt % 128, OFF[j] + tt // 128] = s.astype(np.float32)
                ld_tok[r, base + tt] = j * 128 + s
    return dict(per=per, CA=CA, CB=CB, CACOLS=CACOLS, CBCOLS=CBCOLS,
                COLS=COLS, OFF=OFF, GSW=GSW, GSOFF=GSOFF,
                ICC=ICC), gsi, segid, ld_tok


def _ic_groups(n_chunks, icc, ic_limit=512):
    """Group consecutive chunks for the alpha_d indirect copy; each group
    covers gsz chunks of icc cols each, 16*gsz*icc <= ic_limit."""
    per = ic_limit // (16 * icc)
    groups = []
    pos = 0
    while pos < n_chunks:
        sz = min(per, n_chunks - pos)
        groups.append((pos, sz, sz * icc))
        pos += sz
    return groups


def _build_aidx(ld_tok_r, meta, groups):
    """Build the u16 idx stream for the grouped indirect copies.
    Group covering chunks [c0, c0+gsz) has C_all = gsz*ICC columns;
    IC output stream position i on partition 16*gg + (i%16), col i//16 maps
    to token tt = j*128 + 16*gg + k where k = i//C_all, j = i%C_all
    (each 16-partition group gg covers tokens with residue [16gg, 16gg+16))."""
    icc = meta["ICC"]
    parts = []
    for (c0, gsz, C_all) in groups:
        M = 16 * C_all
        out = np.zeros((128, M // 16), np.uint16)
        i_arr = np.arange(M)
        k_arr = i_arr // C_all
        j_arr = i_arr % C_all
        base = c0 * icc * 128
        ld = ld_tok_r[base:base + 128 * C_all]
        for gg in range(8):
            tt = j_arr * 128 + 16 * gg + k_arr
            out[16 * gg + i_arr % 16, i_arr // 16] = ld[tt].astype(np.uint16)
        parts.append(out)
    return np.concatenate(parts, axis=1)


def _prep_inputs(x, edge_index, batch, Ws, asrcs, adsts, bs):
    src = np.concatenate([edge_index[0], np.arange(N, dtype=np.int64)])
    dst = np.concatenate([edge_index[1], np.arange(N, dtype=np.int64)])
    src = np.asarray(src, np.int64)
    dst = np.asarray(dst, np.int64)

    meta, gsi, segid, ld_tok = _build_edge_data(src, dst)
    groups = _ic_groups(NTILES, meta["ICC"])
    meta["groups"] = groups

    xT_own = np.zeros((NCORES, F, NLOC), np.float32)
    xf = np.asarray(x, np.float32).T
    for r in range(NCORES):
        lo = r * NLOC
        w = min(NLOC, max(0, N - lo))
        xT_own[r, :, :w] = xf[:, lo:lo + w]

    w_aug = np.zeros((3, F, F + 2), np.float32)
    for k in range(3):
        w_aug[k, :, :F] = Ws[k]
        w_aug[k, :, F] = Ws[k] @ asrcs[k]
        w_aug[k, :, F + 1] = Ws[k] @ adsts[k]

    b_rep = np.zeros((3, 128, F), np.float32)
    for k in range(3):
        b_rep[k] = np.tile(bs[k][None, :], (128, 1))

    import ml_dtypes
    iota = np.tile(np.arange(128, dtype=np.float32)[None, :],
                   (128, 1)).astype(ml_dtypes.bfloat16)

    batch64 = np.asarray(batch, np.int64)
    phot = np.zeros((NCORES, NTILES, 128, G), np.float32)
    for r in range(NCORES):
        base = r * NLOC
        for j in range(NTILES):
            nodes = base + j * 128 + np.arange(128)
            valid = nodes < N
            gsel = batch64[np.minimum(nodes, N - 1)]
            ph = np.zeros((128, G), np.float32)
            ph[np.arange(128)[valid], gsel[valid]] = 1.0
            phot[r, j] = ph

    counts = np.bincount(batch64, minlength=G).astype(np.float32)

    in_maps = []
    for r in range(NCORES):
        in_maps.append({
            "xT_own": np.ascontiguousarray(xT_own[r]),
            "w_aug": w_aug,
            "b_rep": b_rep,
            "gsi": np.ascontiguousarray(gsi[r]),
            "segid": np.ascontiguousarray(segid[r]).astype(ml_dtypes.bfloat16),
            "aidx": _build_aidx(ld_tok[r], meta, groups),
            "iota": iota,
            "phot": phot[r].reshape(NTILES * 128, G).astype(ml_dtypes.bfloat16),
        })
    return in_maps, meta, counts


# ---------------- numpy emulation of the device program ----------------

def _emulate_full(in_maps, meta, counts):
    CA, CB = meta["CA"], meta["CB"]
    CACOLS, COLS, OFF, GSOFF = meta["CACOLS"], meta["COLS"], meta["OFF"], meta["GSOFF"]
    hT_cur = [im["xT_own"].copy() for im in in_maps]
    pool_part = [np.zeros((G, F), np.float32) for _ in range(NCORES)]
    for k in range(3):
        # table = allgather of own Wh
        tabs = []
        alss = []
        ad_reps = []
        for r in range(NCORES):
            w = in_maps[r]["w_aug"][k]
            tabs.append((hT_cur[r].T @ w[:, :F]).astype(np.float32))
            alss.append((hT_cur[r].T @ w[:, F]).astype(np.float32))
            ad = (w[:, F + 1][None, :] @ hT_cur[r])[0]
            ad_aug = np.full(NAUG, BIG_NEG, np.float32)
            ad_aug[:NLOC] = ad
            ad_reps.append(ad_aug)
        table = np.concatenate(tabs, axis=0)          # [NPAD, F]
        als_tab = np.concatenate(alss, axis=0)        # [NPAD]
        new_hT = []
        for r in range(NCORES):
            im = in_maps[r]
            b = im["b_rep"][k][0]
            # adt via the aidx emulation (validates _build_aidx)
            ICC = meta["ICC"]
            adt_all = np.zeros((128, NTILES * ICC), np.float32)
            aoff = 0
            for (c0, gsz, C_all) in meta["groups"]:
                M = 16 * C_all
                i_arr = np.arange(M)
                k_arr = i_arr // C_all
                j_arr = i_arr % C_all
                a16 = im["aidx"][:, aoff:aoff + C_all].astype(np.int64)
                aoff += C_all
                for gg in range(8):
                    stream = a16[16 * gg + i_arr % 16, i_arr // 16]
                    vals = ad_reps[r][stream]
                    # stream pos i -> token (j*128 + 16gg + k) of group slots
                    tt = j_arr * 128 + 16 * gg + k_arr
                    adt_all[tt % 128, ICC * c0 + tt // 128] = vals
            ho_all = np.zeros((NLOC, F), np.float32)
            for j in range(NTILES):
                nslots = COLS[j] * 128
                gbuf = np.zeros((nslots, F), np.float32)
                als = np.zeros(nslots, np.float32)
                gs = im["gsi"][:16, GSOFF[j]:GSOFF[j + 1]]
                tA = np.arange(CA[j])
                idxA = gs[tA % 16, tA // 16].astype(np.int64)
                vA = idxA >= 0
                gbuf[tA[vA]] = table[idxA[vA]]
                als[tA[vA]] = als_tab[idxA[vA]]
                tB = np.arange(CB[j])
                idxB = gs[tB % 16, CA[j] // 16 + tB // 16].astype(np.int64)
                vB = idxB >= 0
                gbuf[128 * CACOLS[j] + tB[vB]] = table[BANK + idxB[vB]]
                als[128 * CACOLS[j] + tB[vB]] = als_tab[BANK + idxB[vB]]
                sg = im["segid"][:, OFF[j]:OFF[j + 1]]
                t = np.arange(nslots)
                seg_t = sg[t % 128, t // 128]
                ad_t = adt_all[t % 128, ICC * j + t // 128]
                e = als + ad_t
                e = np.maximum(e, NEG_SLOPE * e)
                p = np.exp(e).astype(np.float32)
                mask = seg_t[:, None] == np.arange(128)[None, :]
                sp = mask * p[:, None]                 # [slots, 128seg]
                out = sp.T @ gbuf                      # [128, F]
                ssum = sp.T @ np.ones(nslots, np.float32)
                h1 = out / (ssum[:, None] + EPS) + b[None, :]
                ho = np.where(h1 > 0, h1, np.exp(np.minimum(h1, 0)) - 1)
                ho_all[128 * j:128 * (j + 1)] = ho.astype(np.float32)
            if k < 2:
                new_hT.append(ho_all.T.copy())
            else:
                ph = im["phot"].reshape(NTILES, 128, G)
                for j in range(NTILES):
                    pool_part[r] += ph[j].T @ ho_all[128 * j:128 * (j + 1)]
        if k < 2:
            hT_cur = new_hT
    total = np.sum(pool_part, axis=0)
    return (total / np.maximum(counts, 1.0)[:, None]).astype(np.float32)


# ---------------- bass program ----------------

def _build_program(meta, repeat=1, nqueues=4, features=("gather", "ic", "cc", "mm")):
    import concourse.bacc as bacc
    import concourse.bass as bass
    import concourse.mybir as mybir
    import concourse.tile as tile
    from concourse import masks

    f32 = mybir.dt.float32
    bf16 = mybir.dt.bfloat16
    i16 = mybir.dt.int16
    u16 = mybir.dt.uint16
    AF = mybir.ActivationFunctionType
    ALU = mybir.AluOpType
    AX = mybir.AxisListType

    CA = [int(v) for v in meta["CA"]]
    CB = [int(v) for v in meta["CB"]]
    CACOLS = [int(v) for v in meta["CACOLS"]]
    COLS = [int(v) for v in meta["COLS"]]
    OFF = [int(v) for v in meta["OFF"]]
    GSOFF = [int(v) for v in meta["GSOFF"]]
    features = set(features)
    groups = meta["groups"]
    ICC = meta["ICC"]
    TOTCOLS = OFF[-1]
    GSTOT = GSOFF[-1]
    AW = sum(g[2] for g in groups)
    CMAX = max(COLS)
    ROWW = 256                # table row width (bf16): [Wh(128)|als|1|pad]
    TBLK = 3                  # table-build tiles per block (PSUM 3*132*4B)

    nc = bacc.Bacc("TRN2", target_bir_lowering=False, debug=False,
                   num_devices=NCORES, num_swdge_queues=nqueues,
                   dynamic_dma_scratch_size=32768)

    # --- dram I/O ---
    xT_own = nc.dram_tensor("xT_own", [F, NLOC], f32, kind="ExternalInput")
    w_aug_d = nc.dram_tensor("w_aug", [3, F, F + 2], f32, kind="ExternalInput")
    b_rep_d = nc.dram_tensor("b_rep", [3, 128, F], f32, kind="ExternalInput")
    gsi_d = nc.dram_tensor("gsi", [128, GSTOT], i16, kind="ExternalInput")
    segid_d = nc.dram_tensor("segid", [128, TOTCOLS], bf16, kind="ExternalInput")
    aidx_d = nc.dram_tensor("aidx", [128, AW], u16, kind="ExternalInput")
    iota_d = nc.dram_tensor("iota", [128, 128], bf16, kind="ExternalInput")
    phot_d = nc.dram_tensor("phot", [NTILES * 128, G], bf16,
                            kind="ExternalInput")
    pool_out = nc.dram_tensor("pool_part", [G, F], f32, kind="ExternalOutput")

    # --- internal dram ---
    cc_in = nc.dram_tensor("cc_in", [NLOC, ROWW], bf16, kind="Internal")
    cc_out = nc.dram_tensor("cc_out", [NCORES, NLOC, ROWW], bf16,
                            kind="Internal", addr_space="Shared")
    table = cc_out.ap().rearrange("r n f -> (r n) f")

    with tile.TileContext(nc) as tc:
        with (
            tc.tile_pool(name="persist", bufs=1) as persist,
            tc.tile_pool(name="gb", bufs=4) as gb_pool,
            tc.tile_pool(name="sp", bufs=3) as sp_pool,
            tc.tile_pool(name="edge", bufs=3) as edge_pool,
            tc.tile_pool(name="post", bufs=3) as post_pool,
            tc.tile_pool(name="pst", bufs=2, space="PSUM") as pst_pool,
            tc.tile_pool(name="ptr", bufs=1, space="PSUM") as ptr_pool,
            tc.tile_pool(name="ps1", bufs=1, space="PSUM") as ps1_pool,
            tc.tile_pool(name="psb", bufs=1, space="PSUM") as psb_pool,
            tc.tile_pool(name="pso", bufs=2, space="PSUM") as pso_pool,
            tc.tile_pool(name="pspool", bufs=1, space="PSUM") as pspool_pool,
        ):
            # persistent tiles
            hT = persist.tile([F, NLOC], f32, tag="hT")
            ad_rep = persist.tile([128, NAUG], f32, tag="ad_rep")
            ad_row = persist.tile([1, NLOC], f32, tag="ad_row")
            adt_all = persist.tile([128, NTILES * ICC], f32, tag="adt_all")
            identity = persist.tile([128, 128], f32, tag="identity")
            ones_col = persist.tile([1, 128], f32, tag="ones_col")
            w_sb = persist.tile([F, F + 2], f32, tag="w_sb")
            b_sb = persist.tile([128, F], bf16, tag="b_sb")
            b_f32 = persist.tile([128, F], f32, tag="b_f32")
            identity_bf = persist.tile([128, 128], bf16, tag="identity_bf")
            ic_out = persist.tile([128, 512], f32, tag="ic_out")
            aidx_sb = persist.tile([128, AW], u16, tag="aidx_sb")
            gsi_sb = persist.tile([128, GSTOT], i16, tag="gsi_sb")
            segid_sb = persist.tile([128, TOTCOLS], bf16, tag="segid_sb")
            iota_sb = persist.tile([128, 128], bf16, tag="iota_sb")
            st_bufs = [persist.tile([128, TBLK, ROWW], bf16, tag=f"st{i}",
                                    name=f"st{i}") for i in range(3)]

            masks.make_identity(nc, identity[:])
            nc.vector.tensor_copy(identity_bf[:], identity[:])
            nc.gpsimd.memset(ones_col[:], 1.0)
            nc.sync.dma_start(aidx_sb[:], aidx_d.ap())
            nc.sync.dma_start(gsi_sb[:], gsi_d.ap())
            nc.sync.dma_start(segid_sb[:], segid_d.ap())
            nc.sync.dma_start(iota_sb[:], iota_d.ap())
            for stb in st_bufs:
                nc.vector.memset(stb[:], 0.0)
                nc.vector.memset(stb[:, :, F + 1:F + 2], 1.0)

            for rep in range(repeat):
              nc.sync.dma_start(hT[:], xT_own.ap())
              for k in range(3):
                  nc.sync.dma_start(w_sb[:], w_aug_d.ap()[k])
                  nc.sync.dma_start(b_f32[:], b_rep_d.ap()[k])
                  nc.vector.tensor_copy(b_sb[:], b_f32[:])

                  # ---- own-shard [Wh|als] -> cc_in (bf16); AllGather = table
                  pos = 0
                  blk = 0
                  while pos < NTILES:
                      nt = min(TBLK, NTILES - pos)
                      ps = pst_pool.tile([128, TBLK, 132], f32, tag="ps_tab")
                      for i in range(nt):
                          nc.tensor.matmul(
                              ps[:, i, 0:F + 1],
                              hT[:, 128 * (pos + i):128 * (pos + i + 1)],
                              w_sb[:, 0:F + 1], start=True, stop=True)
                      st = st_bufs[blk % 3]
                      nc.scalar.activation(st[:, 0:nt, 0:F + 1],
                                           ps[:, 0:nt, 0:F + 1], AF.Copy)
                      dst = cc_in.ap()[128 * pos:128 * (pos + nt)] \
                          .rearrange("(t p) f -> p t f", t=nt)
                      nc.scalar.dma_start(dst, st[:, 0:nt])
                      pos += nt
                      blk += 1
                  if "cc" in features:
                      nc.gpsimd.collective_compute(
                          "AllGather", mybir.AluOpType.bypass,
                          replica_groups=[list(range(NCORES))],
                          ins=[cc_in.ap().opt()], outs=[cc_out.ap().opt()])
                  else:
                      for rr in range(NCORES):
                          nc.sync.dma_start(cc_out.ap()[rr], cc_in.ap()[:])

                  # ---- alpha_d of own nodes -> replicated [128, NAUG] ----
                  ad_chunks = []
                  pos = 0
                  while pos < NLOC:
                      sz = min(512, NLOC - pos)
                      ad_chunks.append((pos, sz))
                      pos += sz
                  for pos, sz in ad_chunks:
                      sl = slice(pos, pos + sz)
                      pr = ps1_pool.tile([1, 512], f32, tag="ps_ad")
                      nc.tensor.matmul(pr[:, 0:sz], w_sb[:, F + 1:F + 2],
                                       hT[:, sl], start=True, stop=True)
                      nc.vector.tensor_copy(ad_row[:, sl], pr[:, 0:sz])
                  for pos, sz in ad_chunks:
                      sl = slice(pos, pos + sz)
                      pb = psb_pool.tile([128, 512], f32, tag="ps_bc")
                      nc.tensor.matmul(pb[:, 0:sz], ones_col[:], ad_row[:, sl],
                                       start=True, stop=True)
                      nc.vector.tensor_copy(ad_rep[:, sl], pb[:, 0:sz])
                  nc.vector.memset(ad_rep[:, NLOC:NAUG], BIG_NEG)

                  # ---- per-token alpha_d via grouped indirect copies ----
                  if "ic" in features:
                      aoff = 0
                      for (c0, gsz, C_all) in groups:
                          M = 16 * C_all
                          nc.gpsimd.indirect_copy(
                              ic_out[:, 0:M], ad_rep[:],
                              aidx_sb[:, aoff:aoff + C_all], True)
                          src_ap = ic_out[:, 0:M].rearrange(
                              "(g o) (kk j) -> g o kk j",
                              g=8, o=16, kk=16, j=C_all)[:, 0]
                          nc.sync.dma_start(
                              adt_all[:, ICC * c0:ICC * c0 + C_all], src_ap)
                          aoff += C_all
                  else:
                      nc.vector.memset(adt_all[:], 0.0)

                  # ---- edge phase: one chunk per dst tile ----
                  # Software-pipelined: chunk j's post-processing is emitted
                  # after chunk j+1's mask/matmuls so DVE never stalls on PE.
                  if k == 2:
                      ps_pl = pspool_pool.tile([G, F], f32, tag="ps_pl")

                  def emit_front(j):
                      cj = COLS[j]
                      gbuf = gb_pool.tile([128, CMAX, ROWW], bf16, tag="gbuf")
                      if "gather" not in features:
                          nc.vector.memset(gbuf[:], 0.0)
                      else:
                          nc.gpsimd.dma_gather(
                              gbuf[:, 0:CACOLS[j]], table[0:BANK],
                              gsi_sb[:, GSOFF[j]:GSOFF[j] + CA[j] // 16],
                              CA[j], CA[j], ROWW, single_packet=False,
                              queue_num=(2 * j) % nqueues)
                          nc.gpsimd.dma_gather(
                              gbuf[:, CACOLS[j]:cj], table[BANK:NPAD],
                              gsi_sb[:, GSOFF[j] + CA[j] // 16:GSOFF[j + 1]],
                              CB[j], CB[j], ROWW, single_packet=False,
                              queue_num=(2 * j + 1) % nqueues)

                      # e = als + alpha_d ; p = exp(leakyrelu(e))
                      e = edge_pool.tile([128, CMAX], f32, tag="e")
                      nc.vector.tensor_tensor(e[:, 0:cj], gbuf[:, 0:cj, F],
                                              adt_all[:, ICC * j:ICC * j + cj],
                                              ALU.add)
                      nc.vector.scalar_tensor_tensor(
                          e[:, 0:cj], e[:, 0:cj], NEG_SLOPE, e[:, 0:cj],
                          ALU.mult, ALU.max)
                      p_bf = edge_pool.tile([128, CMAX], bf16, tag="p_bf")
                      nc.scalar.activation(p_bf[:, 0:cj], e[:, 0:cj], AF.Exp)

                      # S'[token, seg] = p * (segid == seg)
                      sp = sp_pool.tile([128, CMAX, 128], bf16, tag="sp")
                      sg_b = segid_sb[:, OFF[j]:OFF[j + 1]] \
                          .unsqueeze(2).broadcast_to([128, cj, 128])
                      io_b = iota_sb[:].unsqueeze(1).broadcast_to([128, cj, 128])
                      nc.vector.tensor_tensor(sp[:, 0:cj], sg_b, io_b,
                                              ALU.is_equal)
                      p_b = p_bf[:, 0:cj].unsqueeze(2).broadcast_to([128, cj, 128])
                      nc.vector.tensor_tensor(sp[:, 0:cj], sp[:, 0:cj], p_b,
                                              ALU.mult)

                      # segment sums in PSUM: cols [Wh(128)|junk|sum_p]
                      ps_o = pso_pool.tile([128, 132], f32, tag="ps_o")
                      if "mm" in features:
                          for g in range(cj):
                              nc.tensor.matmul(ps_o[:, 0:F + 2], sp[:, g],
                                               gbuf[:, g, 0:F + 2],
                                               start=(g == 0),
                                               stop=(g == cj - 1))
                      else:
                          nc.tensor.matmul(ps_o[:, 0:F + 2], sp[:, 0],
                                           gbuf[:, 0, 0:F + 2],
                                           start=True, stop=True)
                      return ps_o

                  def emit_post(j, ps_o):
                      s_t = post_pool.tile([128, 1], f32, tag="s_t")
                      nc.vector.tensor_scalar_add(s_t[:], ps_o[:, F + 1:F + 2],
                                                  EPS)
                      r_t = post_pool.tile([128, 1], f32, tag="r_t")
                      nc.vector.reciprocal(r_t[:], s_t[:])
                      h1 = post_pool.tile([128, F], bf16, tag="h1")
                      nc.vector.tensor_scalar(h1[:], ps_o[:, 0:F], r_t[:], None,
                                              ALU.mult)
                      nc.vector.tensor_tensor(h1[:], h1[:], b_sb[:], ALU.add)
                      mn = post_pool.tile([128, F], bf16, tag="mn")
                      nc.vector.tensor_scalar_min(mn[:], h1[:], 0.0)
                      ex = post_pool.tile([128, F], bf16, tag="ex")
                      nc.scalar.activation(ex[:], mn[:], AF.Exp)
                      rl = post_pool.tile([128, F], bf16, tag="rl")
                      nc.vector.tensor_scalar_max(rl[:], h1[:], 0.0)
                      ho = post_pool.tile([128, F], bf16, tag="ho")
                      nc.vector.scalar_tensor_tensor(ho[:], ex[:], -1.0, rl[:],
                                                     ALU.add, ALU.add)
                      if k < 2:
                          pt = ptr_pool.tile([128, 128], bf16, tag="ps_tr")
                          nc.tensor.transpose(pt[:], ho[:], identity_bf[:])
                          nc.scalar.activation(hT[:, 128 * j:128 * (j + 1)],
                                               pt[:], AF.Copy)
                      else:
                          ph = post_pool.tile([128, G], bf16, tag="ph")
                          nc.sync.dma_start(
                              ph[:], phot_d.ap()[128 * j:128 * (j + 1)])
                          nc.tensor.matmul(ps_pl[:], ph[:], ho[:],
                                           start=(j == 0),
                                           stop=(j == NTILES - 1))

                  pending = None
                  for j in range(NTILES):
                      ps_o = emit_front(j)
                      if pending is not None:
                          emit_post(pending[0], pending[1])
                      pending = (j, ps_o)
                  emit_post(pending[0], pending[1])

                  if k == 2:
                      pl_sb = post_pool.tile([G, F], f32, tag="pl_sb")
                      nc.vector.tensor_copy(pl_sb[:], ps_pl[:])
                      nc.sync.dma_start(pool_out.ap()[:], pl_sb[:])

    nc.compile()
    return nc


# ---------------- entry point ----------------

LAST_EXEC_NS = None
LAST_META = None


def kernel(x, edge_index, batch,
           W1, a_src1, a_dst1, b1,
           W2, a_src2, a_dst2, b2,
           W3, a_src3, a_dst3, b3):
    global LAST_EXEC_NS, LAST_META
    x = np.asarray(x, np.float32)
    edge_index = np.asarray(edge_index)
    batch = np.asarray(batch)
    Ws = [np.asarray(W1, np.float32), np.asarray(W2, np.float32),
          np.asarray(W3, np.float32)]
    asrcs = [np.asarray(a_src1, np.float32), np.asarray(a_src2, np.float32),
             np.asarray(a_src3, np.float32)]
    adsts = [np.asarray(a_dst1, np.float32), np.asarray(a_dst2, np.float32),
             np.asarray(a_dst3, np.float32)]
    bs = [np.asarray(b1, np.float32), np.asarray(b2, np.float32),
          np.asarray(b3, np.float32)]

    in_maps, meta, counts = _prep_inputs(x, edge_index, batch, Ws, asrcs,
                                         adsts, bs)
    LAST_META = meta

    from concourse.bass_utils import run_bass_kernel_spmd
    nc = _build_program(meta)
    res = run_bass_kernel_spmd(nc, in_maps, core_ids=list(range(NCORES)))
    LAST_EXEC_NS = res.exec_time_ns
    total = np.zeros((G, F), np.float32)
    for r in range(NCORES):
        total += res.results[r]["pool_part"]
    out = total / np.maximum(counts, 1.0)[:, None]
    return out.astype(np.float32)


# revision 37
# speedup vs baseline: 1.2211x; 1.1794x over previous
"""GAT encoder (3-layer) on 8 Trainium2 NeuronCores — scatter-free design.

Sharding: nodes partitioned across cores (graph partition). Edges partitioned
by destination node; weights replicated.

Key design vs the earlier gather+scatter version: the HW profile showed the
GpSimd Q7 core serially generating DMA descriptors for dma_gather AND
dma_scatter_add (~30us per 2048-edge chunk) while the DMA engines idled at
~50%. This version removes the scatter entirely and shrinks the table build:

  1. Per layer, each core computes Wh for ITS OWN nodes only (49 matmuls)
     and the AllGather of those [NLOC, F] bf16 shards IS the gather table
     (node-major [NPAD, F] in shared DRAM). No redundant 392-tile build,
     no separate table store, bf16 rows halve gather bytes.
  2. Edges are grouped by destination TILE (128 consecutive dst nodes), one
     chunk per tile. Segment softmax + scatter-add happen ON-CHIP: a DVE
     iota-compare builds S'[token, seg] = p_token one-hot-weighted, and
     16 PE matmuls accumulate out[seg, :] += S'_g.T @ gbuf_g in PSUM
     (fp32 accumulate). Sum-of-p comes from S'_g.T @ ones. Post-processing
     (divide, bias, ELU, transpose/pool) runs per chunk from PSUM — the
     out_aug HBM round-trip is gone.
  3. Gather idx streams carry trailing -1 pads (trimmed by Q7 before
     descriptor gen, so per-core count variance costs nothing); in-stream
     pads are killed by segid=-1 (S' row = 0) and alpha_d sentinel -1e9.
  4. Gathers cycle over 4 SWDGE queues so one chunk's drain overlaps the
     next chunk's descriptor generation.
"""

import math
import numpy as np

# ---------------- constants (hardcoded problem shape) ----------------
N = 50000
F = 128
G = 64
NCORES = 8
NLOC = 6272                   # 49*128 nodes per core (padded)
NPAD = NLOC * NCORES          # 50176
NTILES = NLOC // 128          # 49 = dst tiles per core = chunks per layer
BANK = 32768                  # gather bank split (int16 idx range)
NAUG = NLOC + 64              # alpha_d replicated width (sentinel tail)
NEG_SLOPE = 0.2
BIG_NEG = -1.0e9
EPS = 1.0e-16
IC_GROUP = 2                  # chunks per indirect-copy call (ISA dst limit 512)
PADFILL = True                # pad gather idx streams with 0 to full width


# ---------------- host-side preprocessing ----------------

def _build_edge_data(src, dst):
    """Group edges by (core, dst-tile, src-bank); build per-chunk gather idx
    streams (wrapped-16, trailing -1 pads), segid arrays, and the alpha_d
    indirect-copy idx stream."""
    per = {}
    for r in range(NCORES):
        lo, hi = r * NLOC, (r + 1) * NLOC
        m = (dst >= lo) & (dst < hi)
        gs = src[m].astype(np.int64)
        ld = (dst[m] - lo).astype(np.int64)
        tile = ld // 128
        seg = ld % 128
        bankB = gs >= BANK
        for j in range(NTILES):
            tm = tile == j
            mA = tm & ~bankB
            mB = tm & bankB
            per[(r, j)] = (gs[mA], seg[mA], gs[mB] - BANK, seg[mB])

    # per-chunk bank widths (max over cores, rounded to 16)
    CA = np.zeros(NTILES, np.int64)
    CB = np.zeros(NTILES, np.int64)
    for j in range(NTILES):
        for r in range(NCORES):
            gA, _, gB, _ = per[(r, j)]
            CA[j] = max(CA[j], len(gA))
            CB[j] = max(CB[j], len(gB))
    CA = ((CA + 15) // 16) * 16
    CB = ((CB + 15) // 16) * 16
    CACOLS = (CA + 127) // 128
    CBCOLS = (CB + 127) // 128
    if PADFILL:
        CA = CACOLS * 128
        CB = CBCOLS * 128
    COLS = CACOLS + CBCOLS                      # slot cols per chunk
    OFF = np.concatenate([[0], np.cumsum(COLS)])  # cumulative col offsets
    GSW = (CA + CB) // 16                        # idx words per chunk
    GSOFF = np.concatenate([[0], np.cumsum(GSW)])

    def wrap16(vals, width):
        """vals (int) -> [16, width//16] wrapped (token t -> [t%16, t//16]),
        then tiled to [128, width//16]."""
        a = np.full(width, 0 if PADFILL else -1, np.int64)
        a[:len(vals)] = vals
        t = np.arange(width)
        w = np.zeros((16, width // 16), np.int16)
        w[t % 16, t // 16] = a.astype(np.int16)
        return np.tile(w, (8, 1))

    # adt IC stream is padded to a fixed ICC=16 cols (2048 slots) per chunk
    # so the grouped indirect copies keep the known-good 512-elem geometry.
    ICC = 16
    assert COLS.max() <= ICC
    gsi = np.zeros((NCORES, 128, GSOFF[-1]), np.int16)
    segid = np.full((NCORES, 128, OFF[-1]), -1.0, np.float32)
    ld_tok = np.full((NCORES, NTILES * ICC * 128), NLOC, np.int64)

    for r in range(NCORES):
        for j in range(NTILES):
            gA, sA, gB, sB = per[(r, j)]
            gsi[r, :, GSOFF[j]:GSOFF[j] + CA[j] // 16] = wrap16(gA, CA[j])
            gsi[r, :, GSOFF[j] + CA[j] // 16:GSOFF[j + 1]] = wrap16(gB, CB[j])
            # slot s (= col*128 + p) -> seg / local dst
            base = j * ICC * 128
            for (g, s, s0) in ((gA, sA, 0), (gB, sB, 128 * CACOLS[j])):
                tt = s0 + np.arange(len(g))
                segid[r, tt % 128, OFF[j] + tt // 128] = s.astype(np.float32)
                ld_tok[r, base + tt] = j * 128 + s
    return dict(per=per, CA=CA, CB=CB, CACOLS=CACOLS, CBCOLS=CBCOLS,
                COLS=COLS, OFF=OFF, GSW=GSW, GSOFF=GSOFF,
                ICC=ICC), gsi, segid, ld_tok


def _ic_groups(n_chunks, icc, ic_limit=512):
    """Group consecutive chunks for the alpha_d indirect copy; each group
    covers gsz chunks of icc cols each, 16*gsz*icc <= ic_limit."""
    per = ic_limit // (16 * icc)
    groups = []
    pos = 0
    while pos < n_chunks:
        sz = min(per, n_chunks - pos)
        groups.append((pos, sz, sz * icc))
        pos += sz
    return groups


def _build_aidx(ld_tok_r, meta, groups):
    """Build the u16 idx stream for the grouped indirect copies.
    Group covering chunks [c0, c0+gsz) has C_all = gsz*ICC columns;
    IC output stream position i on partition 16*gg + (i%16), col i//16 maps
    to token tt = j*128 + 16*gg + k where k = i//C_all, j = i%C_all
    (each 16-partition group gg covers tokens with residue [16gg, 16gg+16))."""
    icc = meta["ICC"]
    parts = []
    for (c0, gsz, C_all) in groups:
        M = 16 * C_all
        out = np.zeros((128, M // 16), np.uint16)
        i_arr = np.arange(M)
        k_arr = i_arr // C_all
        j_arr = i_arr % C_all
        base = c0 * icc * 128
        ld = ld_tok_r[base:base + 128 * C_all]
        for gg in range(8):
            tt = j_arr * 128 + 16 * gg + k_arr
            out[16 * gg + i_arr % 16, i_arr // 16] = ld[tt].astype(np.uint16)
        parts.append(out)
    return np.concatenate(parts, axis=1)


def _prep_inputs(x, edge_index, batch, Ws, asrcs, adsts, bs):
    src = np.concatenate([edge_index[0], np.arange(N, dtype=np.int64)])
    dst = np.concatenate([edge_index[1], np.arange(N, dtype=np.int64)])
    src = np.asarray(src, np.int64)
    dst = np.asarray(dst, np.int64)

    meta, gsi, segid, ld_tok = _build_edge_data(src, dst)
    groups = _ic_groups(NTILES, meta["ICC"])
    meta["groups"] = groups

    xT_own = np.zeros((NCORES, F, NLOC), np.float32)
    xf = np.asarray(x, np.float32).T
    for r in range(NCORES):
        lo = r * NLOC
        w = min(NLOC, max(0, N - lo))
        xT_own[r, :, :w] = xf[:, lo:lo + w]

    w_aug = np.zeros((3, F, F + 2), np.float32)
    for k in range(3):
        w_aug[k, :, :F] = Ws[k]
        w_aug[k, :, F] = Ws[k] @ asrcs[k]
        w_aug[k, :, F + 1] = Ws[k] @ adsts[k]

    b_rep = np.zeros((3, 128, F), np.float32)
    for k in range(3):
        b_rep[k] = np.tile(bs[k][None, :], (128, 1))

    import ml_dtypes
    iota = np.tile(np.arange(128, dtype=np.float32)[None, :],
                   (128, 1)).astype(ml_dtypes.bfloat16)

    batch64 = np.asarray(batch, np.int64)
    phot = np.zeros((NCORES, NTILES, 128, G), np.float32)
    for r in range(NCORES):
        base = r * NLOC
        for j in range(NTILES):
            nodes = base + j * 128 + np.arange(128)
            valid = nodes < N
            gsel = batch64[np.minimum(nodes, N - 1)]
            ph = np.zeros((128, G), np.float32)
            ph[np.arange(128)[valid], gsel[valid]] = 1.0
            phot[r, j] = ph

    counts = np.bincount(batch64, minlength=G).astype(np.float32)

    in_maps = []
    for r in range(NCORES):
        in_maps.append({
            "xT_own": np.ascontiguousarray(xT_own[r]),
            "w_aug": w_aug,
            "b_rep": b_rep,
            "gsi": np.ascontiguousarray(gsi[r]),
            "segid": np.ascontiguousarray(segid[r]).astype(ml_dtypes.bfloat16),
            "aidx": _build_aidx(ld_tok[r], meta, groups),
            "iota": iota,
            "phot": phot[r].reshape(NTILES * 128, G).astype(ml_dtypes.bfloat16),
        })
    return in_maps, meta, counts


# ---------------- numpy emulation of the device program ----------------

def _emulate_full(in_maps, meta, counts):
    CA, CB = meta["CA"], meta["CB"]
    CACOLS, COLS, OFF, GSOFF = meta["CACOLS"], meta["COLS"], meta["OFF"], meta["GSOFF"]
    hT_cur = [im["xT_own"].copy() for im in in_maps]
    pool_part = [np.zeros((G, F), np.float32) for _ in range(NCORES)]
    for k in range(3):
        # table = allgather of own Wh
        tabs = []
        alss = []
        ad_reps = []
        for r in range(NCORES):
            w = in_maps[r]["w_aug"][k]
            tabs.append((hT_cur[r].T @ w[:, :F]).astype(np.float32))
            alss.append((hT_cur[r].T @ w[:, F]).astype(np.float32))
            ad = (w[:, F + 1][None, :] @ hT_cur[r])[0]
            ad_aug = np.full(NAUG, BIG_NEG, np.float32)
            ad_aug[:NLOC] = ad
            ad_reps.append(ad_aug)
        table = np.concatenate(tabs, axis=0)          # [NPAD, F]
        als_tab = np.concatenate(alss, axis=0)        # [NPAD]
        new_hT = []
        for r in range(NCORES):
            im = in_maps[r]
            b = im["b_rep"][k][0]
            # adt via the aidx emulation (validates _build_aidx)
            ICC = meta["ICC"]
            adt_all = np.zeros((128, NTILES * ICC), np.float32)
            aoff = 0
            for (c0, gsz, C_all) in meta["groups"]:
                M = 16 * C_all
                i_arr = np.arange(M)
                k_arr = i_arr // C_all
                j_arr = i_arr % C_all
                a16 = im["aidx"][:, aoff:aoff + C_all].astype(np.int64)
                aoff += C_all
                for gg in range(8):
                    stream = a16[16 * gg + i_arr % 16, i_arr // 16]
                    vals = ad_reps[r][stream]
                    # stream pos i -> token (j*128 + 16gg + k) of group slots
                    tt = j_arr * 128 + 16 * gg + k_arr
                    adt_all[tt % 128, ICC * c0 + tt // 128] = vals
            ho_all = np.zeros((NLOC, F), np.float32)
            for j in range(NTILES):
                nslots = COLS[j] * 128
                gbuf = np.zeros((nslots, F), np.float32)
                als = np.zeros(nslots, np.float32)
                gs = im["gsi"][:16, GSOFF[j]:GSOFF[j + 1]]
                tA = np.arange(CA[j])
                idxA = gs[tA % 16, tA // 16].astype(np.int64)
                vA = idxA >= 0
                gbuf[tA[vA]] = table[idxA[vA]]
                als[tA[vA]] = als_tab[idxA[vA]]
                tB = np.arange(CB[j])
                idxB = gs[tB % 16, CA[j] // 16 + tB // 16].astype(np.int64)
                vB = idxB >= 0
                gbuf[128 * CACOLS[j] + tB[vB]] = table[BANK + idxB[vB]]
                als[128 * CACOLS[j] + tB[vB]] = als_tab[BANK + idxB[vB]]
                sg = im["segid"][:, OFF[j]:OFF[j + 1]]
                t = np.arange(nslots)
                seg_t = sg[t % 128, t // 128]
                ad_t = adt_all[t % 128, ICC * j + t // 128]
                e = als + ad_t
                e = np.maximum(e, NEG_SLOPE * e)
                p = np.exp(e).astype(np.float32)
                mask = seg_t[:, None] == np.arange(128)[None, :]
                sp = mask * p[:, None]                 # [slots, 128seg]
                out = sp.T @ gbuf                      # [128, F]
                ssum = sp.T @ np.ones(nslots, np.float32)
                h1 = out / (ssum[:, None] + EPS) + b[None, :]
                ho = np.where(h1 > 0, h1, np.exp(np.minimum(h1, 0)) - 1)
                ho_all[128 * j:128 * (j + 1)] = ho.astype(np.float32)
            if k < 2:
                new_hT.append(ho_all.T.copy())
            else:
                ph = im["phot"].reshape(NTILES, 128, G)
                for j in range(NTILES):
                    pool_part[r] += ph[j].T @ ho_all[128 * j:128 * (j + 1)]
        if k < 2:
            hT_cur = new_hT
    total = np.sum(pool_part, axis=0)
    return (total / np.maximum(counts, 1.0)[:, None]).astype(np.float32)


# ---------------- bass program ----------------

def _build_program(meta, repeat=1, nqueues=4, features=("gather", "ic", "cc", "mm")):
    import concourse.bacc as bacc
    import concourse.bass as bass
    import concourse.mybir as mybir
    import concourse.tile as tile
    from concourse import masks

    f32 = mybir.dt.float32
    bf16 = mybir.dt.bfloat16
    i16 = mybir.dt.int16
    u16 = mybir.dt.uint16
    AF = mybir.ActivationFunctionType
    ALU = mybir.AluOpType
    AX = mybir.AxisListType

    CA = [int(v) for v in meta["CA"]]
    CB = [int(v) for v in meta["CB"]]
    CACOLS = [int(v) for v in meta["CACOLS"]]
    COLS = [int(v) for v in meta["COLS"]]
    OFF = [int(v) for v in meta["OFF"]]
    GSOFF = [int(v) for v in meta["GSOFF"]]
    features = set(features)
    groups = meta["groups"]
    ICC = meta["ICC"]
    TOTCOLS = OFF[-1]
    GSTOT = GSOFF[-1]
    AW = sum(g[2] for g in groups)
    CMAX = max(COLS)
    ROWW = 256                # table row width (bf16): [Wh(128)|als|1|pad]
    TBLK = 3                  # table-build tiles per block (PSUM 3*132*4B)

    nc = bacc.Bacc("TRN2", target_bir_lowering=False, debug=False,
                   num_devices=NCORES, num_swdge_queues=nqueues,
                   dynamic_dma_scratch_size=32768)

    # --- dram I/O ---
    xT_own = nc.dram_tensor("xT_own", [F, NLOC], f32, kind="ExternalInput")
    w_aug_d = nc.dram_tensor("w_aug", [3, F, F + 2], f32, kind="ExternalInput")
    b_rep_d = nc.dram_tensor("b_rep", [3, 128, F], f32, kind="ExternalInput")
    gsi_d = nc.dram_tensor("gsi", [128, GSTOT], i16, kind="ExternalInput")
    segid_d = nc.dram_tensor("segid", [128, TOTCOLS], bf16, kind="ExternalInput")
    aidx_d = nc.dram_tensor("aidx", [128, AW], u16, kind="ExternalInput")
    iota_d = nc.dram_tensor("iota", [128, 128], bf16, kind="ExternalInput")
    phot_d = nc.dram_tensor("phot", [NTILES * 128, G], bf16,
                            kind="ExternalInput")
    pool_out = nc.dram_tensor("pool_part", [G, F], f32, kind="ExternalOutput")

    # --- internal dram ---
    cc_in = nc.dram_tensor("cc_in", [NLOC, ROWW], bf16, kind="Internal")
    cc_out = nc.dram_tensor("cc_out", [NCORES, NLOC, ROWW], bf16,
                            kind="Internal", addr_space="Shared")
    table = cc_out.ap().rearrange("r n f -> (r n) f")

    with tile.TileContext(nc) as tc:
        with (
            tc.tile_pool(name="persist", bufs=1) as persist,
            tc.tile_pool(name="gb", bufs=6) as gb_pool,
            tc.tile_pool(name="sp", bufs=4) as sp_pool,
            tc.tile_pool(name="edge", bufs=3) as edge_pool,
            tc.tile_pool(name="post", bufs=3) as post_pool,
            tc.tile_pool(name="pst", bufs=2, space="PSUM") as pst_pool,
            tc.tile_pool(name="ptr", bufs=1, space="PSUM") as ptr_pool,
            tc.tile_pool(name="ps1", bufs=1, space="PSUM") as ps1_pool,
            tc.tile_pool(name="psb", bufs=1, space="PSUM") as psb_pool,
            tc.tile_pool(name="pso", bufs=2, space="PSUM") as pso_pool,
            tc.tile_pool(name="pspool", bufs=1, space="PSUM") as pspool_pool,
        ):
            # persistent tiles
            hT = persist.tile([F, NLOC], f32, tag="hT")
            ad_rep = persist.tile([128, NAUG], f32, tag="ad_rep")
            ad_row = persist.tile([1, NLOC], f32, tag="ad_row")
            adt_all = persist.tile([128, NTILES * ICC], f32, tag="adt_all")
            identity = persist.tile([128, 128], f32, tag="identity")
            ones_col = persist.tile([1, 128], f32, tag="ones_col")
            w_sb = persist.tile([F, F + 2], f32, tag="w_sb")
            b_sb = persist.tile([128, F], bf16, tag="b_sb")
            b_f32 = persist.tile([128, F], f32, tag="b_f32")
            identity_bf = persist.tile([128, 128], bf16, tag="identity_bf")
            ic_outs = [persist.tile([128, 512], f32, tag=f"ic_out{i}",
                                    name=f"ic_out{i}") for i in range(3)]
            aidx_sb = persist.tile([128, AW], u16, tag="aidx_sb")
            gsi_sb = persist.tile([128, GSTOT], i16, tag="gsi_sb")
            segid_sb = persist.tile([128, TOTCOLS], bf16, tag="segid_sb")
            iota_sb = persist.tile([128, 128], bf16, tag="iota_sb")
            st_bufs = [persist.tile([128, TBLK, ROWW], bf16, tag=f"st{i}",
                                    name=f"st{i}") for i in range(3)]

            masks.make_identity(nc, identity[:])
            nc.vector.tensor_copy(identity_bf[:], identity[:])
            nc.gpsimd.memset(ones_col[:], 1.0)
            nc.sync.dma_start(aidx_sb[:], aidx_d.ap())
            nc.sync.dma_start(gsi_sb[:], gsi_d.ap())
            nc.sync.dma_start(segid_sb[:], segid_d.ap())
            nc.sync.dma_start(iota_sb[:], iota_d.ap())
            for stb in st_bufs:
                nc.vector.memset(stb[:], 0.0)
                nc.vector.memset(stb[:, :, F + 1:F + 2], 1.0)

            for rep in range(repeat):
              nc.sync.dma_start(hT[:], xT_own.ap())
              for k in range(3):
                  nc.sync.dma_start(w_sb[:], w_aug_d.ap()[k])
                  nc.sync.dma_start(b_f32[:], b_rep_d.ap()[k])
                  nc.vector.tensor_copy(b_sb[:], b_f32[:])

                  # ---- own-shard [Wh|als] -> cc_in (bf16); AllGather = table
                  pos = 0
                  blk = 0
                  while pos < NTILES:
                      nt = min(TBLK, NTILES - pos)
                      ps = pst_pool.tile([128, TBLK, 132], f32, tag="ps_tab")
                      for i in range(nt):
                          nc.tensor.matmul(
                              ps[:, i, 0:F + 1],
                              hT[:, 128 * (pos + i):128 * (pos + i + 1)],
                              w_sb[:, 0:F + 1], start=True, stop=True)
                      st = st_bufs[blk % 3]
                      nc.scalar.activation(st[:, 0:nt, 0:F + 1],
                                           ps[:, 0:nt, 0:F + 1], AF.Copy)
                      dst = cc_in.ap()[128 * pos:128 * (pos + nt)] \
                          .rearrange("(t p) f -> p t f", t=nt)
                      nc.scalar.dma_start(dst, st[:, 0:nt])
                      pos += nt
                      blk += 1
                  if "cc" in features:
                      nc.gpsimd.collective_compute(
                          "AllGather", mybir.AluOpType.bypass,
                          replica_groups=[list(range(NCORES))],
                          ins=[cc_in.ap().opt()], outs=[cc_out.ap().opt()])
                  else:
                      for rr in range(NCORES):
                          nc.sync.dma_start(cc_out.ap()[rr], cc_in.ap()[:])

                  # ---- alpha_d of own nodes -> replicated [128, NAUG] ----
                  ad_chunks = []
                  pos = 0
                  while pos < NLOC:
                      sz = min(512, NLOC - pos)
                      ad_chunks.append((pos, sz))
                      pos += sz
                  for pos, sz in ad_chunks:
                      sl = slice(pos, pos + sz)
                      pr = ps1_pool.tile([1, 512], f32, tag="ps_ad")
                      nc.tensor.matmul(pr[:, 0:sz], w_sb[:, F + 1:F + 2],
                                       hT[:, sl], start=True, stop=True)
                      nc.scalar.activation(ad_row[:, sl], pr[:, 0:sz], AF.Copy)
                  for pos, sz in ad_chunks:
                      sl = slice(pos, pos + sz)
                      pb = psb_pool.tile([128, 512], f32, tag="ps_bc")
                      nc.tensor.matmul(pb[:, 0:sz], ones_col[:], ad_row[:, sl],
                                       start=True, stop=True)
                      nc.scalar.activation(ad_rep[:, sl], pb[:, 0:sz], AF.Copy)
                  nc.vector.memset(ad_rep[:, NLOC:NAUG], BIG_NEG)

                  # ---- per-token alpha_d via grouped indirect copies ----
                  if "ic" in features:
                      aoff = 0
                      for gi_, (c0, gsz, C_all) in enumerate(groups):
                          M = 16 * C_all
                          ic_out = ic_outs[gi_ % 3]
                          nc.gpsimd.indirect_copy(
                              ic_out[:, 0:M], ad_rep[:],
                              aidx_sb[:, aoff:aoff + C_all], True)
                          src_ap = ic_out[:, 0:M].rearrange(
                              "(g o) (kk j) -> g o kk j",
                              g=8, o=16, kk=16, j=C_all)[:, 0]
                          nc.sync.dma_start(
                              adt_all[:, ICC * c0:ICC * c0 + C_all], src_ap)
                          aoff += C_all
                  else:
                      nc.vector.memset(adt_all[:], 0.0)

                  # ---- edge phase: one chunk per dst tile ----
                  # Software-pipelined: chunk j's post-processing is emitted
                  # after chunk j+1's mask/matmuls so DVE never stalls on PE.
                  if k == 2:
                      ps_pl = pspool_pool.tile([G, F], f32, tag="ps_pl")

                  def emit_front(j):
                      cj = COLS[j]
                      gbuf = gb_pool.tile([128, CMAX, ROWW], bf16, tag="gbuf")
                      if "gather" not in features:
                          nc.vector.memset(gbuf[:], 0.0)
                      else:
                          nc.gpsimd.dma_gather(
                              gbuf[:, 0:CACOLS[j]], table[0:BANK],
                              gsi_sb[:, GSOFF[j]:GSOFF[j] + CA[j] // 16],
                              CA[j], CA[j], ROWW, single_packet=False,
                              queue_num=(2 * j) % nqueues)
                          nc.gpsimd.dma_gather(
                              gbuf[:, CACOLS[j]:cj], table[BANK:NPAD],
                              gsi_sb[:, GSOFF[j] + CA[j] // 16:GSOFF[j + 1]],
                              CB[j], CB[j], ROWW, single_packet=False,
                              queue_num=(2 * j + 1) % nqueues)

                      # e = als + alpha_d ; p = exp(leakyrelu(e))
                      e = edge_pool.tile([128, CMAX], f32, tag="e")
                      nc.vector.tensor_tensor(e[:, 0:cj], gbuf[:, 0:cj, F],
                                              adt_all[:, ICC * j:ICC * j + cj],
                                              ALU.add)
                      nc.vector.scalar_tensor_tensor(
                          e[:, 0:cj], e[:, 0:cj], NEG_SLOPE, e[:, 0:cj],
                          ALU.mult, ALU.max)
                      p_bf = edge_pool.tile([128, CMAX], bf16, tag="p_bf")
                      nc.scalar.activation(p_bf[:, 0:cj], e[:, 0:cj], AF.Exp)

                      # S'[token, seg] = p * (segid == seg)
                      sp = sp_pool.tile([128, CMAX, 128], bf16, tag="sp")
                      sg_b = segid_sb[:, OFF[j]:OFF[j + 1]] \
                          .unsqueeze(2).broadcast_to([128, cj, 128])
                      io_b = iota_sb[:].unsqueeze(1).broadcast_to([128, cj, 128])
                      nc.vector.tensor_tensor(sp[:, 0:cj], sg_b, io_b,
                                              ALU.is_equal)
                      p_b = p_bf[:, 0:cj].unsqueeze(2).broadcast_to([128, cj, 128])
                      nc.vector.tensor_tensor(sp[:, 0:cj], sp[:, 0:cj], p_b,
                                              ALU.mult)

                      # segment sums in PSUM: cols [Wh(128)|junk|sum_p]
                      ps_o = pso_pool.tile([128, 132], f32, tag="ps_o")
                      if "mm" in features:
                          for g in range(cj):
                              nc.tensor.matmul(ps_o[:, 0:F + 2], sp[:, g],
                                               gbuf[:, g, 0:F + 2],
                                               start=(g == 0),
                                               stop=(g == cj - 1))
                      else:
                          nc.tensor.matmul(ps_o[:, 0:F + 2], sp[:, 0],
                                           gbuf[:, 0, 0:F + 2],
                                           start=True, stop=True)
                      return ps_o

                  def emit_post(j, ps_o):
                      s_t = post_pool.tile([128, 1], f32, tag="s_t")
                      nc.vector.tensor_scalar_add(s_t[:], ps_o[:, F + 1:F + 2],
                                                  EPS)
                      r_t = post_pool.tile([128, 1], f32, tag="r_t")
                      nc.vector.reciprocal(r_t[:], s_t[:])
                      h1 = post_pool.tile([128, F], bf16, tag="h1")
                      nc.vector.tensor_scalar(h1[:], ps_o[:, 0:F], r_t[:], None,
                                              ALU.mult)
                      nc.vector.tensor_tensor(h1[:], h1[:], b_sb[:], ALU.add)
                      mn = post_pool.tile([128, F], bf16, tag="mn")
                      nc.vector.tensor_scalar_min(mn[:], h1[:], 0.0)
                      ex = post_pool.tile([128, F], bf16, tag="ex")
                      nc.scalar.activation(ex[:], mn[:], AF.Exp)
                      rl = post_pool.tile([128, F], bf16, tag="rl")
                      nc.vector.tensor_scalar_max(rl[:], h1[:], 0.0)
                      ho = post_pool.tile([128, F], bf16, tag="ho")
                      nc.vector.scalar_tensor_tensor(ho[:], ex[:], -1.0, rl[:],
                                                     ALU.add, ALU.add)
                      if k < 2:
                          pt = ptr_pool.tile([128, 128], bf16, tag="ps_tr")
                          nc.tensor.transpose(pt[:], ho[:], identity_bf[:])
                          nc.scalar.activation(hT[:, 128 * j:128 * (j + 1)],
                                               pt[:], AF.Copy)
                      else:
                          ph = post_pool.tile([128, G], bf16, tag="ph")
                          nc.sync.dma_start(
                              ph[:], phot_d.ap()[128 * j:128 * (j + 1)])
                          nc.tensor.matmul(ps_pl[:], ph[:], ho[:],
                                           start=(j == 0),
                                           stop=(j == NTILES - 1))

                  pending = None
                  for j in range(NTILES):
                      ps_o = emit_front(j)
                      if pending is not None:
                          emit_post(pending[0], pending[1])
                      pending = (j, ps_o)
                  emit_post(pending[0], pending[1])

                  if k == 2:
                      pl_sb = post_pool.tile([G, F], f32, tag="pl_sb")
                      nc.vector.tensor_copy(pl_sb[:], ps_pl[:])
                      nc.sync.dma_start(pool_out.ap()[:], pl_sb[:])

    nc.compile()
    return nc


# ---------------- entry point ----------------

LAST_EXEC_NS = None
LAST_META = None


def kernel(x, edge_index, batch,
           W1, a_src1, a_dst1, b1,
           W2, a_src2, a_dst2, b2,
           W3, a_src3, a_dst3, b3):
    global LAST_EXEC_NS, LAST_META
    x = np.asarray(x, np.float32)
    edge_index = np.asarray(edge_index)
    batch = np.asarray(batch)
    Ws = [np.asarray(W1, np.float32), np.asarray(W2, np.float32),
          np.asarray(W3, np.float32)]
    asrcs = [np.asarray(a_src1, np.float32), np.asarray(a_src2, np.float32),
             np.asarray(a_src3, np.float32)]
    adsts = [np.asarray(a_dst1, np.float32), np.asarray(a_dst2, np.float32),
             np.asarray(a_dst3, np.float32)]
    bs = [np.asarray(b1, np.float32), np.asarray(b2, np.float32),
          np.asarray(b3, np.float32)]

    in_maps, meta, counts = _prep_inputs(x, edge_index, batch, Ws, asrcs,
                                         adsts, bs)
    LAST_META = meta

    from concourse.bass_utils import run_bass_kernel_spmd
    nc = _build_program(meta)
    res = run_bass_kernel_spmd(nc, in_maps, core_ids=list(range(NCORES)))
    LAST_EXEC_NS = res.exec_time_ns
    total = np.zeros((G, F), np.float32)
    for r in range(NCORES):
        total += res.results[r]["pool_part"]
    out = total / np.maximum(counts, 1.0)[:, None]
    return out.astype(np.float32)
